# revision 6
# baseline (speedup 1.0000x reference)
"""Trainium2 kernel for nn_ConnectivityLoss (MALIS connectivity loss).

Contract: kernel(y_true, y_pred) -> scalar float32 loss, matching

    loss = sum(y_pred**2 * weights_n + (20 - y_pred)**2 * weights_p)

where weights_n / weights_p are the MALIS maximin edge weights computed per
32x32 window.  The reference itself computes the weights with a host
``jax.pure_callback`` (sequential Kruskal + union-find is not device work);
this kernel mirrors that split: a fast bit-exact host implementation of the
weights, and the memory-bound weighted reduction on 8 NeuronCores
(data-parallel over the flattened pixels).
"""

import numpy as np

# ===========================================================================
# Host side: bit-exact fast reimplementation of the reference MALIS weights.
# ===========================================================================

WIN = 32
_N = WIN * WIN
_idx = np.arange(_N).reshape(WIN, WIN)
E1 = np.concatenate([_idx[:, :-1].ravel(), _idx[:-1, :].ravel()]).astype(np.int64)
E2 = np.concatenate([_idx[:, 1:].ravel(), _idx[1:, :].ravel()]).astype(np.int64)
_E = E1.shape[0]  # 1984
_HALF = WIN * (WIN - 1)  # 992

try:
    from scipy import ndimage as _ndi

    def _label_bg(bg):  # bg: bool (WIN, WIN) -> int32 labels, 0 = unlabeled
        lab, _ = _ndi.label(bg)
        return lab.astype(np.int32)
except Exception:  # pragma: no cover

    def _label_bg(bg):
        lab = np.zeros((WIN, WIN), np.int32)
        nxt = 0
        stack = []
        for i in range(WIN):
            for j in range(WIN):
                if bg[i, j] and lab[i, j] == 0:
                    nxt += 1
                    stack.append((i, j))
                    lab[i, j] = nxt
                    while stack:
                        a, b = stack.pop()
                        for u, v in ((a - 1, b), (a + 1, b), (a, b - 1), (a, b + 1)):
                            if 0 <= u < WIN and 0 <= v < WIN and bg[u, v] and lab[u, v] == 0:
                                lab[u, v] = nxt
                                stack.append((u, v))
        return lab


def _malis_runs_py(orders, segs, e1, e2, pos):
    """Pure-python fallback: Kruskal maximin MALIS weighting, R runs."""
    R, E = orders.shape
    N = segs.shape[1]
    w = np.zeros((R, E), np.int64)
    for r in range(R):
        seg = segs[r]
        parent = np.arange(N, dtype=np.int64)
        cnts = [({int(seg[i]): 1} if seg[i] != 0 else {}) for i in range(N)]
        tot = [len(c) for c in cnts]
        wr = w[r]
        order = orders[r]
        for k in range(E):
            e = order[k]
            a = e1[e]
            while parent[a] != a:
                parent[a] = parent[parent[a]]
                a = parent[a]
            b = e2[e]
            while parent[b] != b:
                parent[b] = parent[parent[b]]
                b = parent[b]
            if a == b:
                continue
            ca, cb = cnts[a], cnts[b]
            if len(ca) > len(cb):
                a, b, ca, cb = b, a, cb, ca
            common = 0
            for l, c in ca.items():
                if l in cb:
                    common += c * cb[l]
            if pos:
                wr[e] = common
            else:
                wr[e] = tot[a] * tot[b] - common
            for l, c in ca.items():
                cb[l] = cb.get(l, 0) + c
            tot[b] += tot[a]
            parent[a] = b
            cnts[a] = {}
    return w


_malis_runs = None
try:
    import numba

    @numba.njit(cache=True)
    def _malis_runs_nb(orders, segs, e1, e2, pos):
        R, E = orders.shape
        N = segs.shape[1]
        w = np.zeros((R, E), np.int64)
        for r in range(R):
            seg = segs[r]
            L = 0
            for i in range(N):
                if seg[i] > L:
                    L = seg[i]
            parent = np.arange(N, dtype=np.int64)
            cnt = np.zeros((N, L + 1), np.int64)
            tot = np.zeros(N, np.int64)
            for i in range(N):
                if seg[i] != 0:
                    cnt[i, seg[i]] = 1
                    tot[i] = 1
            for k in range(E):
                e = orders[r, k]
                a = e1[e]
                while parent[a] != a:
                    parent[a] = parent[parent[a]]
                    a = parent[a]
                b = e2[e]
                while parent[b] != b:
                    parent[b] = parent[parent[b]]
                    b = parent[b]
                if a == b:
                    continue
                common = np.int64(0)
                for l in range(1, L + 1):
                    common += cnt[a, l] * cnt[b, l]
                if pos:
                    w[r, e] = common
                else:
                    w[r, e] = tot[a] * tot[b] - common
                for l in range(1, L + 1):
                    cnt[b, l] += cnt[a, l]
                tot[b] += tot[a]
                parent[a] = b
        return w

    _malis_runs = _malis_runs_nb
except Exception:  # pragma: no cover
    _malis_runs = None


def malis_weights_full(pred, target):
    """Bit-exact clone of the reference's _malis_weights_full."""
    pred = np.ascontiguousarray(np.asarray(pred, dtype=np.float32))
    target = np.ascontiguousarray(np.asarray(target, dtype=np.float32))
    B, C, H, W = pred.shape
    nR, nC = H // WIN, W // WIN
    R = B * nR * nC

    p = pred[:, 0].reshape(B, nR, WIN, nC, WIN).transpose(0, 1, 3, 2, 4)
    t = target[:, 0].reshape(B, nR, WIN, nC, WIN).transpose(0, 1, 3, 2, 4)

    costs_h = (p[..., :, :-1] + p[..., :, 1:]).reshape(B, nR, nC, _HALF)
    costs_v = (p[..., :-1, :] + p[..., 1:, :]).reshape(B, nR, nC, _HALF)
    costs = np.concatenate([costs_h, costs_v], axis=-1)  # (B,nR,nC,E) f32
    gt_h = (t[..., :, :-1] + t[..., :, 1:]).reshape(B, nR, nC, _HALF)
    gt_v = (t[..., :-1, :] + t[..., 1:, :]).reshape(B, nR, nC, _HALF)
    gt = np.concatenate([gt_h, gt_v], axis=-1)

    costs_n = costs.copy()
    costs_p = costs.copy()
    costs_n[gt > 20] = 20
    costs_p[gt < 10] = 0
    gtc = np.minimum(gt, 20)

    # stable descending argsort — identical tie-breaking to the reference
    order_n = np.ascontiguousarray(
        np.argsort(-costs_n, axis=-1, kind="stable").reshape(R, _E)
    )
    order_p = np.ascontiguousarray(
        np.argsort(-costs_p, axis=-1, kind="stable").reshape(R, _E)
    )

    segs = np.empty((B, nR, nC, _N), np.int32)
    bg = t == 0.0
    for b in range(B):
        for r in range(nR):
            for c in range(nC):
                segs[b, r, c] = _label_bg(bg[b, r, c]).ravel()
    segs2 = segs.reshape(R, _N)

    global _malis_runs
    if _malis_runs is not None:
        try:
            wn = _malis_runs(order_n, segs2, E1, E2, 0)
            wp = _malis_runs(order_p, segs2, E1, E2, 1)
        except Exception:
            _malis_runs = None
            wn = _malis_runs_py(order_n, segs2, E1, E2, 0)
            wp = _malis_runs_py(order_p, segs2, E1, E2, 1)
    else:
        wn = _malis_runs_py(order_n, segs2, E1, E2, 0)
        wp = _malis_runs_py(order_p, segs2, E1, E2, 1)

    out = []
    gtc_flat = gtc.reshape(R, _E)
    for w, is_pos in ((wn, False), (wp, True)):
        w64 = w.astype(np.float64)
        s = w64.sum(axis=-1, keepdims=True)
        np.divide(w64, s, out=w64, where=s > 0)
        if is_pos:
            w64[gtc_flat < 20] = 0
        else:
            w64[gtc_flat >= 10] = 0
        wh = w64[:, :_HALF].reshape(R, WIN, WIN - 1)
        wv = w64[:, _HALF:].reshape(R, WIN - 1, WIN)
        nw = np.zeros((R, WIN, WIN), np.float64)
        nw[:, :, :-1] += wh
        nw[:, :, 1:] += wh
        nw[:, :-1, :] += wv
        nw[:, 1:, :] += wv
        img = (
            nw.reshape(B, nR, nC, WIN, WIN)
            .transpose(0, 1, 3, 2, 4)
            .reshape(B, 1, H, W)
            .astype(np.float32)
        )
        out.append(img)
    return out[0], out[1]


# ===========================================================================
# Device side: weighted-loss reduction on 8 NeuronCores.
# ===========================================================================

N_CORES = 8
_P = 128  # SBUF partitions
_TOT = 4 * 1 * 256 * 256  # 262144 pixels
_PER_CORE = _TOT // N_CORES  # 32768
_F = _PER_CORE // _P  # 256 floats per partition per tensor

_NC_CACHE = {}


def _build_nc():
    import concourse.bacc as bacc
    import concourse.tile as tile
    from concourse import mybir

    f32 = mybir.dt.float32
    nc = bacc.Bacc("TRN2", target_bir_lowering=False)
    yp_d = nc.dram_tensor("yp", [_P, _F], f32, kind="ExternalInput")
    wn_d = nc.dram_tensor("wn", [_P, _F], f32, kind="ExternalInput")
    wp_d = nc.dram_tensor("wp", [_P, _F], f32, kind="ExternalInput")
    out_d = nc.dram_tensor("out", [_P, 1], f32, kind="ExternalOutput")

    with tile.TileContext(nc) as tc:
        with tc.tile_pool(name="io", bufs=1) as io:
            t_yp = io.tile([_P, _F], f32)
            t_wn = io.tile([_P, _F], f32)
            t_wp = io.tile([_P, _F], f32)
            nc.sync.dma_start(out=t_yp[:, :], in_=yp_d[:, :])
            nc.sync.dma_start(out=t_wn[:, :], in_=wn_d[:, :])
            nc.sync.dma_start(out=t_wp[:, :], in_=wp_d[:, :])

            # loss = yp^2 * wn + (yp-20)^2 * wp, reduced over the free axis
            t_sq1 = io.tile([_P, _F], f32)
            nc.vector.tensor_mul(t_sq1[:, :], t_yp[:, :], t_yp[:, :])
            t_m1 = io.tile([_P, _F], f32)
            nc.vector.tensor_mul(t_m1[:, :], t_sq1[:, :], t_wn[:, :])
            t_b = io.tile([_P, _F], f32)
            nc.vector.tensor_scalar_sub(t_b[:, :], t_yp[:, :], 20.0)
            t_sq2 = io.tile([_P, _F], f32)
            nc.vector.tensor_mul(t_sq2[:, :], t_b[:, :], t_b[:, :])
            t_m2 = io.tile([_P, _F], f32)
            nc.vector.tensor_mul(t_m2[:, :], t_sq2[:, :], t_wp[:, :])
            t_sum = io.tile([_P, _F], f32)
            nc.vector.tensor_add(t_sum[:, :], t_m1[:, :], t_m2[:, :])
            p2 = io.tile([_P, 1], f32)
            nc.vector.reduce_sum(
                out=p2[:, :], in_=t_sum[:, :], axis=mybir.AxisListType.X
            )
            nc.sync.dma_start(out=out_d[:, :], in_=p2[:, :])
    nc.finalize()
    return nc


def _get_nc():
    if "nc" not in _NC_CACHE:
        _NC_CACHE["nc"] = _build_nc()
    return _NC_CACHE["nc"]


def _shard(arr):
    """(4,1,256,256) f32 -> list of 8 [128, 256] per-core chunks."""
    flat = np.ascontiguousarray(arr, dtype=np.float32).reshape(N_CORES, _P, _F)
    return [flat[c] for c in range(N_CORES)]


def run_device(y_pred, wn_img, wp_img, trace=False, **kw):
    from concourse.bass_utils import run_bass_kernel_spmd

    nc = _get_nc()
    yps = _shard(y_pred)
    wns = _shard(wn_img)
    wps = _shard(wp_img)
    in_maps = [
        {"yp": yps[c], "wn": wns[c], "wp": wps[c]} for c in range(N_CORES)
    ]
    res = run_bass_kernel_spmd(nc, in_maps, core_ids=list(range(N_CORES)), trace=trace, **kw)
    partials = np.concatenate([res.results[c]["out"].ravel() for c in range(N_CORES)])
    total = np.float32(partials.astype(np.float64).sum())
    return total, res


def kernel(y_true, y_pred):
    y_true = np.asarray(y_true, dtype=np.float32)
    y_pred = np.asarray(y_pred, dtype=np.float32)
    wn_img, wp_img = malis_weights_full(y_pred, y_true)
    total, _ = run_device(y_pred, wn_img, wp_img, trace=False)
    return np.array(total, dtype=np.float32)


# revision 9
# speedup vs baseline: 1.4496x; 1.4496x over previous
"""Trainium2 kernel for nn_ConnectivityLoss (MALIS connectivity loss).

Contract: kernel(y_true, y_pred) -> scalar float32 loss, matching

    loss = sum(y_pred**2 * weights_n + (20 - y_pred)**2 * weights_p)

where weights_n / weights_p are the MALIS maximin edge weights computed per
32x32 window.  The reference itself computes the weights with a host
``jax.pure_callback`` (sequential Kruskal + union-find is not device work);
this kernel mirrors that split: a fast bit-exact host implementation of the
weights, and the memory-bound weighted reduction on 8 NeuronCores
(data-parallel over the flattened pixels).
"""

import numpy as np

# ===========================================================================
# Host side: bit-exact fast reimplementation of the reference MALIS weights.
# ===========================================================================

WIN = 32
_N = WIN * WIN
_idx = np.arange(_N).reshape(WIN, WIN)
E1 = np.concatenate([_idx[:, :-1].ravel(), _idx[:-1, :].ravel()]).astype(np.int64)
E2 = np.concatenate([_idx[:, 1:].ravel(), _idx[1:, :].ravel()]).astype(np.int64)
_E = E1.shape[0]  # 1984
_HALF = WIN * (WIN - 1)  # 992

try:
    from scipy import ndimage as _ndi

    def _label_bg(bg):  # bg: bool (WIN, WIN) -> int32 labels, 0 = unlabeled
        lab, _ = _ndi.label(bg)
        return lab.astype(np.int32)
except Exception:  # pragma: no cover

    def _label_bg(bg):
        lab = np.zeros((WIN, WIN), np.int32)
        nxt = 0
        stack = []
        for i in range(WIN):
            for j in range(WIN):
                if bg[i, j] and lab[i, j] == 0:
                    nxt += 1
                    stack.append((i, j))
                    lab[i, j] = nxt
                    while stack:
                        a, b = stack.pop()
                        for u, v in ((a - 1, b), (a + 1, b), (a, b - 1), (a, b + 1)):
                            if 0 <= u < WIN and 0 <= v < WIN and bg[u, v] and lab[u, v] == 0:
                                lab[u, v] = nxt
                                stack.append((u, v))
        return lab


def _malis_runs_py(orders, segs, e1, e2, pos):
    """Pure-python fallback: Kruskal maximin MALIS weighting, R runs."""
    R, E = orders.shape
    N = segs.shape[1]
    w = np.zeros((R, E), np.int64)
    for r in range(R):
        seg = segs[r]
        parent = np.arange(N, dtype=np.int64)
        cnts = [({int(seg[i]): 1} if seg[i] != 0 else {}) for i in range(N)]
        tot = [len(c) for c in cnts]
        wr = w[r]
        order = orders[r]
        for k in range(E):
            e = order[k]
            a = e1[e]
            while parent[a] != a:
                parent[a] = parent[parent[a]]
                a = parent[a]
            b = e2[e]
            while parent[b] != b:
                parent[b] = parent[parent[b]]
                b = parent[b]
            if a == b:
                continue
            ca, cb = cnts[a], cnts[b]
            if len(ca) > len(cb):
                a, b, ca, cb = b, a, cb, ca
            common = 0
            for l, c in ca.items():
                if l in cb:
                    common += c * cb[l]
            if pos:
                wr[e] = common
            else:
                wr[e] = tot[a] * tot[b] - common
            for l, c in ca.items():
                cb[l] = cb.get(l, 0) + c
            tot[b] += tot[a]
            parent[a] = b
            cnts[a] = {}
    return w


_malis_runs = None
try:
    import numba

    @numba.njit(cache=True)
    def _malis_runs_nb(orders, segs, e1, e2, pos):
        R, E = orders.shape
        N = segs.shape[1]
        w = np.zeros((R, E), np.int64)
        for r in range(R):
            seg = segs[r]
            L = 0
            for i in range(N):
                if seg[i] > L:
                    L = seg[i]
            parent = np.arange(N, dtype=np.int64)
            cnt = np.zeros((N, L + 1), np.int64)
            tot = np.zeros(N, np.int64)
            for i in range(N):
                if seg[i] != 0:
                    cnt[i, seg[i]] = 1
                    tot[i] = 1
            for k in range(E):
                e = orders[r, k]
                a = e1[e]
                while parent[a] != a:
                    parent[a] = parent[parent[a]]
                    a = parent[a]
                b = e2[e]
                while parent[b] != b:
                    parent[b] = parent[parent[b]]
                    b = parent[b]
                if a == b:
                    continue
                common = np.int64(0)
                for l in range(1, L + 1):
                    common += cnt[a, l] * cnt[b, l]
                if pos:
                    w[r, e] = common
                else:
                    w[r, e] = tot[a] * tot[b] - common
                for l in range(1, L + 1):
                    cnt[b, l] += cnt[a, l]
                tot[b] += tot[a]
                parent[a] = b
        return w

    _malis_runs = _malis_runs_nb
except Exception:  # pragma: no cover
    _malis_runs = None


def malis_weights_full(pred, target):
    """Bit-exact clone of the reference's _malis_weights_full."""
    pred = np.ascontiguousarray(np.asarray(pred, dtype=np.float32))
    target = np.ascontiguousarray(np.asarray(target, dtype=np.float32))
    B, C, H, W = pred.shape
    nR, nC = H // WIN, W // WIN
    R = B * nR * nC

    p = pred[:, 0].reshape(B, nR, WIN, nC, WIN).transpose(0, 1, 3, 2, 4)
    t = target[:, 0].reshape(B, nR, WIN, nC, WIN).transpose(0, 1, 3, 2, 4)

    costs_h = (p[..., :, :-1] + p[..., :, 1:]).reshape(B, nR, nC, _HALF)
    costs_v = (p[..., :-1, :] + p[..., 1:, :]).reshape(B, nR, nC, _HALF)
    costs = np.concatenate([costs_h, costs_v], axis=-1)  # (B,nR,nC,E) f32
    gt_h = (t[..., :, :-1] + t[..., :, 1:]).reshape(B, nR, nC, _HALF)
    gt_v = (t[..., :-1, :] + t[..., 1:, :]).reshape(B, nR, nC, _HALF)
    gt = np.concatenate([gt_h, gt_v], axis=-1)

    costs_n = costs.copy()
    costs_p = costs.copy()
    costs_n[gt > 20] = 20
    costs_p[gt < 10] = 0
    gtc = np.minimum(gt, 20)

    # stable descending argsort — identical tie-breaking to the reference
    order_n = np.ascontiguousarray(
        np.argsort(-costs_n, axis=-1, kind="stable").reshape(R, _E)
    )
    order_p = np.ascontiguousarray(
        np.argsort(-costs_p, axis=-1, kind="stable").reshape(R, _E)
    )

    segs = np.empty((B, nR, nC, _N), np.int32)
    bg = t == 0.0
    for b in range(B):
        for r in range(nR):
            for c in range(nC):
                segs[b, r, c] = _label_bg(bg[b, r, c]).ravel()
    segs2 = segs.reshape(R, _N)

    global _malis_runs
    if _malis_runs is not None:
        try:
            wn = _malis_runs(order_n, segs2, E1, E2, 0)
            wp = _malis_runs(order_p, segs2, E1, E2, 1)
        except Exception:
            _malis_runs = None
            wn = _malis_runs_py(order_n, segs2, E1, E2, 0)
            wp = _malis_runs_py(order_p, segs2, E1, E2, 1)
    else:
        wn = _malis_runs_py(order_n, segs2, E1, E2, 0)
        wp = _malis_runs_py(order_p, segs2, E1, E2, 1)

    out = []
    gtc_flat = gtc.reshape(R, _E)
    for w, is_pos in ((wn, False), (wp, True)):
        w64 = w.astype(np.float64)
        s = w64.sum(axis=-1, keepdims=True)
        np.divide(w64, s, out=w64, where=s > 0)
        if is_pos:
            w64[gtc_flat < 20] = 0
        else:
            w64[gtc_flat >= 10] = 0
        wh = w64[:, :_HALF].reshape(R, WIN, WIN - 1)
        wv = w64[:, _HALF:].reshape(R, WIN - 1, WIN)
        nw = np.zeros((R, WIN, WIN), np.float64)
        nw[:, :, :-1] += wh
        nw[:, :, 1:] += wh
        nw[:, :-1, :] += wv
        nw[:, 1:, :] += wv
        img = (
            nw.reshape(B, nR, nC, WIN, WIN)
            .transpose(0, 1, 3, 2, 4)
            .reshape(B, 1, H, W)
            .astype(np.float32)
        )
        out.append(img)
    return out[0], out[1]


# ===========================================================================
# Device side: weighted-loss reduction on 8 NeuronCores.
# ===========================================================================

N_CORES = 8
_P = 128  # SBUF partitions
_TOT = 4 * 1 * 256 * 256  # 262144 pixels
_PER_CORE = _TOT // N_CORES  # 32768
_F = _PER_CORE // _P  # 256 floats per partition per tensor

_NC_CACHE = {}


def _build_nc():
    import concourse.bacc as bacc
    import concourse.tile as tile
    from concourse import mybir

    f32 = mybir.dt.float32
    nc = bacc.Bacc("TRN2", target_bir_lowering=False)
    yp_d = nc.dram_tensor("yp", [_P, _F], f32, kind="ExternalInput")
    wn_d = nc.dram_tensor("wn", [_P, _F], f32, kind="ExternalInput")
    wp_d = nc.dram_tensor("wp", [_P, _F], f32, kind="ExternalInput")
    out_d = nc.dram_tensor("out", [1, 1], f32, kind="ExternalOutput")

    with tile.TileContext(nc) as tc:
        with (
            tc.tile_pool(name="io", bufs=1) as io,
            tc.tile_pool(name="ps", bufs=1, space="PSUM") as ps,
        ):
            # three input loads on three different engine queues (parallel)
            t_yp = io.tile([_P, _F], f32)
            t_wn = io.tile([_P, _F], f32)
            t_wp = io.tile([_P, _F], f32)
            nc.sync.dma_start(out=t_yp[:, :], in_=yp_d[:, :])
            nc.scalar.dma_start(out=t_wn[:, :], in_=wn_d[:, :])
            nc.gpsimd.dma_start(out=t_wp[:, :], in_=wp_d[:, :])

            ones = io.tile([_P, 1], f32)
            nc.gpsimd.memset(ones[:, :], 1.0)

            # DVE chain; yp-only ops first so they overlap the wn/wp loads
            t_sq1 = io.tile([_P, _F], f32)
            nc.vector.tensor_mul(t_sq1[:, :], t_yp[:, :], t_yp[:, :])
            t_b = io.tile([_P, _F], f32)
            nc.vector.tensor_scalar_sub(t_b[:, :], t_yp[:, :], 20.0)
            t_sq2 = io.tile([_P, _F], f32)
            nc.vector.tensor_mul(t_sq2[:, :], t_b[:, :], t_b[:, :])
            t_m1 = io.tile([_P, _F], f32)
            nc.vector.tensor_mul(t_m1[:, :], t_sq1[:, :], t_wn[:, :])
            t_m2 = io.tile([_P, _F], f32)
            nc.vector.tensor_mul(t_m2[:, :], t_sq2[:, :], t_wp[:, :])

            # column sums via PE: psum[1,256] = ones.T @ m1 + ones.T @ m2
            col = ps.tile([1, _F], f32)
            nc.tensor.matmul(col[:, :], ones[:, :], t_m1[:, :], start=True, stop=False)
            nc.tensor.matmul(col[:, :], ones[:, :], t_m2[:, :], start=False, stop=True)

            # final reduce [1,256] -> [1,1] and 4-byte store
            t_out = io.tile([1, 1], f32)
            nc.vector.reduce_sum(
                out=t_out[:, :], in_=col[:, :], axis=mybir.AxisListType.X
            )
            nc.sync.dma_start(out=out_d[:, :], in_=t_out[:, :])
    nc.finalize()
    return nc


def _get_nc():
    if "nc" not in _NC_CACHE:
        _NC_CACHE["nc"] = _build_nc()
    return _NC_CACHE["nc"]


def _shard(arr):
    """(4,1,256,256) f32 -> list of 8 [128, 256] per-core chunks."""
    flat = np.ascontiguousarray(arr, dtype=np.float32).reshape(N_CORES, _P, _F)
    return [flat[c] for c in range(N_CORES)]


def run_device(y_pred, wn_img, wp_img, trace=False, **kw):
    from concourse.bass_utils import run_bass_kernel_spmd

    nc = _get_nc()
    yps = _shard(y_pred)
    wns = _shard(wn_img)
    wps = _shard(wp_img)
    in_maps = [
        {"yp": yps[c], "wn": wns[c], "wp": wps[c]} for c in range(N_CORES)
    ]
    res = run_bass_kernel_spmd(nc, in_maps, core_ids=list(range(N_CORES)), trace=trace, **kw)
    partials = np.array(
        [float(res.results[c]["out"][0, 0]) for c in range(N_CORES)], dtype=np.float64
    )
    total = np.float32(partials.sum())
    return total, res


def kernel(y_true, y_pred):
    y_true = np.asarray(y_true, dtype=np.float32)
    y_pred = np.asarray(y_pred, dtype=np.float32)
    wn_img, wp_img = malis_weights_full(y_pred, y_true)
    total, _ = run_device(y_pred, wn_img, wp_img, trace=False)
    return np.array(total, dtype=np.float32)


# revision 12
# speedup vs baseline: 1.4766x; 1.0186x over previous
"""Trainium2 kernel for nn_ConnectivityLoss (MALIS connectivity loss).

Contract: kernel(y_true, y_pred) -> scalar float32 loss, matching

    loss = sum(y_pred**2 * weights_n + (20 - y_pred)**2 * weights_p)

where weights_n / weights_p are the MALIS maximin edge weights computed per
32x32 window.  The reference itself computes the weights with a host
``jax.pure_callback`` (sequential Kruskal + union-find is not device work);
this kernel mirrors that split: a fast bit-exact host implementation of the
weights, and the memory-bound weighted reduction on 8 NeuronCores
(data-parallel over the flattened pixels).
"""

import numpy as np

# ===========================================================================
# Host side: bit-exact fast reimplementation of the reference MALIS weights.
# ===========================================================================

WIN = 32
_N = WIN * WIN
_idx = np.arange(_N).reshape(WIN, WIN)
E1 = np.concatenate([_idx[:, :-1].ravel(), _idx[:-1, :].ravel()]).astype(np.int64)
E2 = np.concatenate([_idx[:, 1:].ravel(), _idx[1:, :].ravel()]).astype(np.int64)
_E = E1.shape[0]  # 1984
_HALF = WIN * (WIN - 1)  # 992

try:
    from scipy import ndimage as _ndi

    def _label_bg(bg):  # bg: bool (WIN, WIN) -> int32 labels, 0 = unlabeled
        lab, _ = _ndi.label(bg)
        return lab.astype(np.int32)
except Exception:  # pragma: no cover

    def _label_bg(bg):
        lab = np.zeros((WIN, WIN), np.int32)
        nxt = 0
        stack = []
        for i in range(WIN):
            for j in range(WIN):
                if bg[i, j] and lab[i, j] == 0:
                    nxt += 1
                    stack.append((i, j))
                    lab[i, j] = nxt
                    while stack:
                        a, b = stack.pop()
                        for u, v in ((a - 1, b), (a + 1, b), (a, b - 1), (a, b + 1)):
                            if 0 <= u < WIN and 0 <= v < WIN and bg[u, v] and lab[u, v] == 0:
                                lab[u, v] = nxt
                                stack.append((u, v))
        return lab


def _malis_runs_py(orders, segs, e1, e2, pos):
    """Pure-python fallback: Kruskal maximin MALIS weighting, R runs."""
    R, E = orders.shape
    N = segs.shape[1]
    w = np.zeros((R, E), np.int64)
    for r in range(R):
        seg = segs[r]
        parent = np.arange(N, dtype=np.int64)
        cnts = [({int(seg[i]): 1} if seg[i] != 0 else {}) for i in range(N)]
        tot = [len(c) for c in cnts]
        wr = w[r]
        order = orders[r]
        for k in range(E):
            e = order[k]
            a = e1[e]
            while parent[a] != a:
                parent[a] = parent[parent[a]]
                a = parent[a]
            b = e2[e]
            while parent[b] != b:
                parent[b] = parent[parent[b]]
                b = parent[b]
            if a == b:
                continue
            ca, cb = cnts[a], cnts[b]
            if len(ca) > len(cb):
                a, b, ca, cb = b, a, cb, ca
            common = 0
            for l, c in ca.items():
                if l in cb:
                    common += c * cb[l]
            if pos:
                wr[e] = common
            else:
                wr[e] = tot[a] * tot[b] - common
            for l, c in ca.items():
                cb[l] = cb.get(l, 0) + c
            tot[b] += tot[a]
            parent[a] = b
            cnts[a] = {}
    return w


_malis_runs = None
try:
    import numba

    @numba.njit(cache=True)
    def _malis_runs_nb(orders, segs, e1, e2, pos):
        R, E = orders.shape
        N = segs.shape[1]
        w = np.zeros((R, E), np.int64)
        for r in range(R):
            seg = segs[r]
            L = 0
            for i in range(N):
                if seg[i] > L:
                    L = seg[i]
            parent = np.arange(N, dtype=np.int64)
            cnt = np.zeros((N, L + 1), np.int64)
            tot = np.zeros(N, np.int64)
            for i in range(N):
                if seg[i] != 0:
                    cnt[i, seg[i]] = 1
                    tot[i] = 1
            for k in range(E):
                e = orders[r, k]
                a = e1[e]
                while parent[a] != a:
                    parent[a] = parent[parent[a]]
                    a = parent[a]
                b = e2[e]
                while parent[b] != b:
                    parent[b] = parent[parent[b]]
                    b = parent[b]
                if a == b:
                    continue
                common = np.int64(0)
                for l in range(1, L + 1):
                    common += cnt[a, l] * cnt[b, l]
                if pos:
                    w[r, e] = common
                else:
                    w[r, e] = tot[a] * tot[b] - common
                for l in range(1, L + 1):
                    cnt[b, l] += cnt[a, l]
                tot[b] += tot[a]
                parent[a] = b
        return w

    _malis_runs = _malis_runs_nb
except Exception:  # pragma: no cover
    _malis_runs = None


def malis_weights_full(pred, target):
    """Bit-exact clone of the reference's _malis_weights_full."""
    pred = np.ascontiguousarray(np.asarray(pred, dtype=np.float32))
    target = np.ascontiguousarray(np.asarray(target, dtype=np.float32))
    B, C, H, W = pred.shape
    nR, nC = H // WIN, W // WIN
    R = B * nR * nC

    p = pred[:, 0].reshape(B, nR, WIN, nC, WIN).transpose(0, 1, 3, 2, 4)
    t = target[:, 0].reshape(B, nR, WIN, nC, WIN).transpose(0, 1, 3, 2, 4)

    costs_h = (p[..., :, :-1] + p[..., :, 1:]).reshape(B, nR, nC, _HALF)
    costs_v = (p[..., :-1, :] + p[..., 1:, :]).reshape(B, nR, nC, _HALF)
    costs = np.concatenate([costs_h, costs_v], axis=-1)  # (B,nR,nC,E) f32
    gt_h = (t[..., :, :-1] + t[..., :, 1:]).reshape(B, nR, nC, _HALF)
    gt_v = (t[..., :-1, :] + t[..., 1:, :]).reshape(B, nR, nC, _HALF)
    gt = np.concatenate([gt_h, gt_v], axis=-1)

    costs_n = costs.copy()
    costs_p = costs.copy()
    costs_n[gt > 20] = 20
    costs_p[gt < 10] = 0
    gtc = np.minimum(gt, 20)

    # stable descending argsort — identical tie-breaking to the reference
    order_n = np.ascontiguousarray(
        np.argsort(-costs_n, axis=-1, kind="stable").reshape(R, _E)
    )
    order_p = np.ascontiguousarray(
        np.argsort(-costs_p, axis=-1, kind="stable").reshape(R, _E)
    )

    segs = np.empty((B, nR, nC, _N), np.int32)
    bg = t == 0.0
    for b in range(B):
        for r in range(nR):
            for c in range(nC):
                segs[b, r, c] = _label_bg(bg[b, r, c]).ravel()
    segs2 = segs.reshape(R, _N)

    global _malis_runs
    if _malis_runs is not None:
        try:
            wn = _malis_runs(order_n, segs2, E1, E2, 0)
            wp = _malis_runs(order_p, segs2, E1, E2, 1)
        except Exception:
            _malis_runs = None
            wn = _malis_runs_py(order_n, segs2, E1, E2, 0)
            wp = _malis_runs_py(order_p, segs2, E1, E2, 1)
    else:
        wn = _malis_runs_py(order_n, segs2, E1, E2, 0)
        wp = _malis_runs_py(order_p, segs2, E1, E2, 1)

    out = []
    gtc_flat = gtc.reshape(R, _E)
    for w, is_pos in ((wn, False), (wp, True)):
        w64 = w.astype(np.float64)
        s = w64.sum(axis=-1, keepdims=True)
        np.divide(w64, s, out=w64, where=s > 0)
        if is_pos:
            w64[gtc_flat < 20] = 0
        else:
            w64[gtc_flat >= 10] = 0
        wh = w64[:, :_HALF].reshape(R, WIN, WIN - 1)
        wv = w64[:, _HALF:].reshape(R, WIN - 1, WIN)
        nw = np.zeros((R, WIN, WIN), np.float64)
        nw[:, :, :-1] += wh
        nw[:, :, 1:] += wh
        nw[:, :-1, :] += wv
        nw[:, 1:, :] += wv
        img = (
            nw.reshape(B, nR, nC, WIN, WIN)
            .transpose(0, 1, 3, 2, 4)
            .reshape(B, 1, H, W)
            .astype(np.float32)
        )
        out.append(img)
    return out[0], out[1]


# ===========================================================================
# Device side: weighted-loss reduction on 8 NeuronCores.
# ===========================================================================

N_CORES = 8
_P = 128  # SBUF partitions
_TOT = 4 * 1 * 256 * 256  # 262144 pixels
_PER_CORE = _TOT // N_CORES  # 32768
_F = _PER_CORE // _P  # 256 floats per partition per tensor

_NC_CACHE = {}


def _build_nc():
    import concourse.bacc as bacc
    import concourse.tile as tile
    from concourse import mybir

    f32 = mybir.dt.float32
    nc = bacc.Bacc("TRN2", target_bir_lowering=False)
    yp_d = nc.dram_tensor("yp", [_P, _F], f32, kind="ExternalInput")
    wn_d = nc.dram_tensor("wn", [_P, _F], f32, kind="ExternalInput")
    wp_d = nc.dram_tensor("wp", [_P, _F], f32, kind="ExternalInput")
    out_d = nc.dram_tensor("out", [1, 1], f32, kind="ExternalOutput")

    with tile.TileContext(nc) as tc:
        with (
            tc.tile_pool(name="io", bufs=1) as io,
            tc.tile_pool(name="ps", bufs=1, space="PSUM") as ps,
        ):
            # three input loads on three different engine queues (parallel)
            t_yp = io.tile([_P, _F], f32)
            t_wn = io.tile([_P, _F], f32)
            t_wp = io.tile([_P, _F], f32)
            nc.sync.dma_start(out=t_yp[:, :], in_=yp_d[:, :])
            nc.scalar.dma_start(out=t_wn[:, :], in_=wn_d[:, :])
            nc.gpsimd.dma_start(out=t_wp[:, :], in_=wp_d[:, :])

            ones = io.tile([_P, 1], f32)
            nc.gpsimd.memset(ones[:, :], 1.0)

            # DVE chain; yp-only ops first so they overlap the wn/wp loads
            t_sq1 = io.tile([_P, _F], f32)
            nc.vector.tensor_mul(t_sq1[:, :], t_yp[:, :], t_yp[:, :])
            t_b = io.tile([_P, _F], f32)
            nc.vector.tensor_scalar_sub(t_b[:, :], t_yp[:, :], 20.0)
            t_sq2 = io.tile([_P, _F], f32)
            nc.vector.tensor_mul(t_sq2[:, :], t_b[:, :], t_b[:, :])
            t_m1 = io.tile([_P, _F], f32)
            nc.vector.tensor_mul(t_m1[:, :], t_sq1[:, :], t_wn[:, :])
            t_m2 = io.tile([_P, _F], f32)
            nc.vector.tensor_mul(t_m2[:, :], t_sq2[:, :], t_wp[:, :])

            # column sums via PE: psum[1,256] = ones.T @ m1 + ones.T @ m2
            col = ps.tile([1, _F], f32)
            nc.tensor.matmul(col[:, :], ones[:, :], t_m1[:, :], start=True, stop=False)
            nc.tensor.matmul(col[:, :], ones[:, :], t_m2[:, :], start=False, stop=True)

            # final reduce [1,256] -> [1,1] and 4-byte store
            t_out = io.tile([1, 1], f32)
            nc.vector.reduce_sum(
                out=t_out[:, :], in_=col[:, :], axis=mybir.AxisListType.X
            )
            nc.sync.dma_start(out=out_d[:, :], in_=t_out[:, :])
    nc.finalize()
    return nc


def _build_nc_raw():
    """Raw Bacc kernel with manual semaphores — skips the Tile framework's
    entry/exit all-engine barriers, ordering modes, and extra prologue."""
    from contextlib import ExitStack

    import concourse.bacc as bacc
    from concourse import mybir

    f32 = mybir.dt.float32
    nc = bacc.Bacc("TRN2", target_bir_lowering=False)
    yp_d = nc.dram_tensor("yp", [_P, _F], f32, kind="ExternalInput")
    wn_d = nc.dram_tensor("wn", [_P, _F], f32, kind="ExternalInput")
    wp_d = nc.dram_tensor("wp", [_P, _F], f32, kind="ExternalInput")
    out_d = nc.dram_tensor("out", [1, 1], f32, kind="ExternalOutput")

    with ExitStack() as ctx:
        t_yp = ctx.enter_context(nc.sbuf_tensor([_P, _F], f32))
        t_wn = ctx.enter_context(nc.sbuf_tensor([_P, _F], f32))
        t_wp = ctx.enter_context(nc.sbuf_tensor([_P, _F], f32))
        t_sq1 = ctx.enter_context(nc.sbuf_tensor([_P, _F], f32))
        t_b = ctx.enter_context(nc.sbuf_tensor([_P, _F], f32))
        t_sq2 = ctx.enter_context(nc.sbuf_tensor([_P, _F], f32))
        t_m1 = ctx.enter_context(nc.sbuf_tensor([_P, _F], f32))
        t_m2 = ctx.enter_context(nc.sbuf_tensor([_P, _F], f32))
        ones = ctx.enter_context(nc.sbuf_tensor([_P, 1], f32))
        p_a = ctx.enter_context(nc.sbuf_tensor([_P, 1], f32))
        p_b = ctx.enter_context(nc.sbuf_tensor([_P, 1], f32))
        p_t = ctx.enter_context(nc.sbuf_tensor([_P, 1], f32))
        t_out = ctx.enter_context(nc.sbuf_tensor([1, 1], f32))
        acc = ctx.enter_context(nc.psum_tensor([1, 1], f32))

        s_yp = ctx.enter_context(nc.semaphore("s_yp"))
        s_wn = ctx.enter_context(nc.semaphore("s_wn"))
        s_wp = ctx.enter_context(nc.semaphore("s_wp"))
        s_ones = ctx.enter_context(nc.semaphore("s_ones"))
        s_p = ctx.enter_context(nc.semaphore("s_p"))
        s_mm = ctx.enter_context(nc.semaphore("s_mm"))
        s_res = ctx.enter_context(nc.semaphore("s_res"))
        s_out = ctx.enter_context(nc.semaphore("s_out"))
        s_v = ctx.enter_context(nc.semaphore("s_v"))
        block = ctx.enter_context(nc.Block())

        @block.sync
        def _(sync):
            sync.dma_start(out=t_yp[:, :], in_=yp_d[:, :]).then_inc(s_yp, 16)
            sync.wait_ge(s_res, 1)
            sync.dma_start(out=out_d[:, :], in_=t_out[:, :]).then_inc(s_out, 16)
            sync.wait_ge(s_out, 16)

        @block.scalar
        def _(scalar):
            scalar.dma_start(out=t_wn[:, :], in_=wn_d[:, :]).then_inc(s_wn, 16)

        @block.gpsimd
        def _(gpsimd):
            gpsimd.memset(ones[:, :], 1.0).then_inc(s_ones, 1)
            gpsimd.dma_start(out=t_wp[:, :], in_=wp_d[:, :]).then_inc(s_wp, 16)

        @block.vector
        def _(vector):
            # s_v serializes the DVE RAW chains (the engine pipeline does
            # not interlock same-engine SBUF read-after-write).
            n = [0]

            def step(ins):
                n[0] += 1
                ins.then_inc(s_v, 1)

            def fence():
                vector.wait_ge(s_v, n[0])

            vector.wait_ge(s_yp, 16)
            step(nc.vector.tensor_mul(t_sq1[:, :], t_yp[:, :], t_yp[:, :]))
            step(nc.vector.tensor_scalar_sub(t_b[:, :], t_yp[:, :], 20.0))
            fence()
            step(nc.vector.tensor_mul(t_sq2[:, :], t_b[:, :], t_b[:, :]))
            vector.wait_ge(s_wn, 16)
            fence()
            step(nc.vector.tensor_mul(t_m1[:, :], t_sq1[:, :], t_wn[:, :]))
            fence()
            step(
                nc.vector.reduce_sum(
                    out=p_a[:, :], in_=t_m1[:, :], axis=mybir.AxisListType.X
                )
            )
            vector.wait_ge(s_wp, 16)
            step(nc.vector.tensor_mul(t_m2[:, :], t_sq2[:, :], t_wp[:, :]))
            fence()
            step(
                nc.vector.reduce_sum(
                    out=p_b[:, :], in_=t_m2[:, :], axis=mybir.AxisListType.X
                )
            )
            fence()
            nc.vector.tensor_add(p_t[:, :], p_a[:, :], p_b[:, :]).then_inc(s_p, 1)
            vector.wait_ge(s_mm, 1)
            nc.vector.tensor_copy(t_out[:, :], acc[:, :]).then_inc(s_res, 1)

        @block.tensor
        def _(tensor):
            tensor.wait_ge(s_p, 1)
            tensor.wait_ge(s_ones, 1)
            nc.tensor.matmul(
                acc[:, :], p_t[:, :], ones[:, :], start=True, stop=True
            ).then_inc(s_mm, 1)

    nc.finalize()
    return nc


def _get_nc():
    if "nc" not in _NC_CACHE:
        import os

        builder = _build_nc_raw if os.environ.get("K_IMPL", "raw") == "raw" else _build_nc
        _NC_CACHE["nc"] = builder()
    return _NC_CACHE["nc"]


def _shard(arr):
    """(4,1,256,256) f32 -> list of 8 [128, 256] per-core chunks."""
    flat = np.ascontiguousarray(arr, dtype=np.float32).reshape(N_CORES, _P, _F)
    return [flat[c] for c in range(N_CORES)]


def run_device(y_pred, wn_img, wp_img, trace=False, **kw):
    from concourse.bass_utils import run_bass_kernel_spmd

    nc = _get_nc()
    yps = _shard(y_pred)
    wns = _shard(wn_img)
    wps = _shard(wp_img)
    in_maps = [
        {"yp": yps[c], "wn": wns[c], "wp": wps[c]} for c in range(N_CORES)
    ]
    res = run_bass_kernel_spmd(nc, in_maps, core_ids=list(range(N_CORES)), trace=trace, **kw)
    partials = np.array(
        [float(res.results[c]["out"][0, 0]) for c in range(N_CORES)], dtype=np.float64
    )
    total = np.float32(partials.sum())
    return total, res


def kernel(y_true, y_pred):
    y_true = np.asarray(y_true, dtype=np.float32)
    y_pred = np.asarray(y_pred, dtype=np.float32)
    wn_img, wp_img = malis_weights_full(y_pred, y_true)
    total, _ = run_device(y_pred, wn_img, wp_img, trace=False)
    return np.array(total, dtype=np.float32)


# revision 15
# speedup vs baseline: 1.5998x; 1.0834x over previous
"""Trainium2 kernel for nn_ConnectivityLoss (MALIS connectivity loss).

Contract: kernel(y_true, y_pred) -> scalar float32 loss, matching

    loss = sum(y_pred**2 * weights_n + (20 - y_pred)**2 * weights_p)

where weights_n / weights_p are the MALIS maximin edge weights computed per
32x32 window.  The reference itself computes the weights with a host
``jax.pure_callback`` (sequential Kruskal + union-find is not device work);
this kernel mirrors that split: a fast bit-exact host implementation of the
weights, and the memory-bound weighted reduction on 8 NeuronCores
(data-parallel over the flattened pixels).
"""

import numpy as np

# ===========================================================================
# Host side: bit-exact fast reimplementation of the reference MALIS weights.
# ===========================================================================

WIN = 32
_N = WIN * WIN
_idx = np.arange(_N).reshape(WIN, WIN)
E1 = np.concatenate([_idx[:, :-1].ravel(), _idx[:-1, :].ravel()]).astype(np.int64)
E2 = np.concatenate([_idx[:, 1:].ravel(), _idx[1:, :].ravel()]).astype(np.int64)
_E = E1.shape[0]  # 1984
_HALF = WIN * (WIN - 1)  # 992

try:
    from scipy import ndimage as _ndi

    def _label_bg(bg):  # bg: bool (WIN, WIN) -> int32 labels, 0 = unlabeled
        lab, _ = _ndi.label(bg)
        return lab.astype(np.int32)
except Exception:  # pragma: no cover

    def _label_bg(bg):
        lab = np.zeros((WIN, WIN), np.int32)
        nxt = 0
        stack = []
        for i in range(WIN):
            for j in range(WIN):
                if bg[i, j] and lab[i, j] == 0:
                    nxt += 1
                    stack.append((i, j))
                    lab[i, j] = nxt
                    while stack:
                        a, b = stack.pop()
                        for u, v in ((a - 1, b), (a + 1, b), (a, b - 1), (a, b + 1)):
                            if 0 <= u < WIN and 0 <= v < WIN and bg[u, v] and lab[u, v] == 0:
                                lab[u, v] = nxt
                                stack.append((u, v))
        return lab


def _malis_runs_py(orders, segs, e1, e2, pos):
    """Pure-python fallback: Kruskal maximin MALIS weighting, R runs."""
    R, E = orders.shape
    N = segs.shape[1]
    w = np.zeros((R, E), np.int64)
    for r in range(R):
        seg = segs[r]
        parent = np.arange(N, dtype=np.int64)
        cnts = [({int(seg[i]): 1} if seg[i] != 0 else {}) for i in range(N)]
        tot = [len(c) for c in cnts]
        wr = w[r]
        order = orders[r]
        for k in range(E):
            e = order[k]
            a = e1[e]
            while parent[a] != a:
                parent[a] = parent[parent[a]]
                a = parent[a]
            b = e2[e]
            while parent[b] != b:
                parent[b] = parent[parent[b]]
                b = parent[b]
            if a == b:
                continue
            ca, cb = cnts[a], cnts[b]
            if len(ca) > len(cb):
                a, b, ca, cb = b, a, cb, ca
            common = 0
            for l, c in ca.items():
                if l in cb:
                    common += c * cb[l]
            if pos:
                wr[e] = common
            else:
                wr[e] = tot[a] * tot[b] - common
            for l, c in ca.items():
                cb[l] = cb.get(l, 0) + c
            tot[b] += tot[a]
            parent[a] = b
            cnts[a] = {}
    return w


_malis_runs = None
try:
    import numba

    @numba.njit(cache=True)
    def _malis_runs_nb(orders, segs, e1, e2, pos):
        R, E = orders.shape
        N = segs.shape[1]
        w = np.zeros((R, E), np.int64)
        for r in range(R):
            seg = segs[r]
            L = 0
            for i in range(N):
                if seg[i] > L:
                    L = seg[i]
            parent = np.arange(N, dtype=np.int64)
            cnt = np.zeros((N, L + 1), np.int64)
            tot = np.zeros(N, np.int64)
            for i in range(N):
                if seg[i] != 0:
                    cnt[i, seg[i]] = 1
                    tot[i] = 1
            for k in range(E):
                e = orders[r, k]
                a = e1[e]
                while parent[a] != a:
                    parent[a] = parent[parent[a]]
                    a = parent[a]
                b = e2[e]
                while parent[b] != b:
                    parent[b] = parent[parent[b]]
                    b = parent[b]
                if a == b:
                    continue
                common = np.int64(0)
                for l in range(1, L + 1):
                    common += cnt[a, l] * cnt[b, l]
                if pos:
                    w[r, e] = common
                else:
                    w[r, e] = tot[a] * tot[b] - common
                for l in range(1, L + 1):
                    cnt[b, l] += cnt[a, l]
                tot[b] += tot[a]
                parent[a] = b
        return w

    _malis_runs = _malis_runs_nb
except Exception:  # pragma: no cover
    _malis_runs = None


def malis_weights_full(pred, target):
    """Bit-exact clone of the reference's _malis_weights_full."""
    pred = np.ascontiguousarray(np.asarray(pred, dtype=np.float32))
    target = np.ascontiguousarray(np.asarray(target, dtype=np.float32))
    B, C, H, W = pred.shape
    nR, nC = H // WIN, W // WIN
    R = B * nR * nC

    p = pred[:, 0].reshape(B, nR, WIN, nC, WIN).transpose(0, 1, 3, 2, 4)
    t = target[:, 0].reshape(B, nR, WIN, nC, WIN).transpose(0, 1, 3, 2, 4)

    costs_h = (p[..., :, :-1] + p[..., :, 1:]).reshape(B, nR, nC, _HALF)
    costs_v = (p[..., :-1, :] + p[..., 1:, :]).reshape(B, nR, nC, _HALF)
    costs = np.concatenate([costs_h, costs_v], axis=-1)  # (B,nR,nC,E) f32
    gt_h = (t[..., :, :-1] + t[..., :, 1:]).reshape(B, nR, nC, _HALF)
    gt_v = (t[..., :-1, :] + t[..., 1:, :]).reshape(B, nR, nC, _HALF)
    gt = np.concatenate([gt_h, gt_v], axis=-1)

    costs_n = costs.copy()
    costs_p = costs.copy()
    costs_n[gt > 20] = 20
    costs_p[gt < 10] = 0
    gtc = np.minimum(gt, 20)

    # stable descending argsort — identical tie-breaking to the reference
    order_n = np.ascontiguousarray(
        np.argsort(-costs_n, axis=-1, kind="stable").reshape(R, _E)
    )
    order_p = np.ascontiguousarray(
        np.argsort(-costs_p, axis=-1, kind="stable").reshape(R, _E)
    )

    segs = np.empty((B, nR, nC, _N), np.int32)
    bg = t == 0.0
    for b in range(B):
        for r in range(nR):
            for c in range(nC):
                segs[b, r, c] = _label_bg(bg[b, r, c]).ravel()
    segs2 = segs.reshape(R, _N)

    global _malis_runs
    if _malis_runs is not None:
        try:
            wn = _malis_runs(order_n, segs2, E1, E2, 0)
            wp = _malis_runs(order_p, segs2, E1, E2, 1)
        except Exception:
            _malis_runs = None
            wn = _malis_runs_py(order_n, segs2, E1, E2, 0)
            wp = _malis_runs_py(order_p, segs2, E1, E2, 1)
    else:
        wn = _malis_runs_py(order_n, segs2, E1, E2, 0)
        wp = _malis_runs_py(order_p, segs2, E1, E2, 1)

    out = []
    gtc_flat = gtc.reshape(R, _E)
    for w, is_pos in ((wn, False), (wp, True)):
        w64 = w.astype(np.float64)
        s = w64.sum(axis=-1, keepdims=True)
        np.divide(w64, s, out=w64, where=s > 0)
        if is_pos:
            w64[gtc_flat < 20] = 0
        else:
            w64[gtc_flat >= 10] = 0
        wh = w64[:, :_HALF].reshape(R, WIN, WIN - 1)
        wv = w64[:, _HALF:].reshape(R, WIN - 1, WIN)
        nw = np.zeros((R, WIN, WIN), np.float64)
        nw[:, :, :-1] += wh
        nw[:, :, 1:] += wh
        nw[:, :-1, :] += wv
        nw[:, 1:, :] += wv
        img = (
            nw.reshape(B, nR, nC, WIN, WIN)
            .transpose(0, 1, 3, 2, 4)
            .reshape(B, 1, H, W)
            .astype(np.float32)
        )
        out.append(img)
    return out[0], out[1]


# ===========================================================================
# Device side: weighted-loss reduction on 8 NeuronCores.
# ===========================================================================

N_CORES = 8
_P = 128  # SBUF partitions
_TOT = 4 * 1 * 256 * 256  # 262144 pixels
_PER_CORE = _TOT // N_CORES  # 32768
_F = _PER_CORE // _P  # 256 floats per partition per tensor

_NC_CACHE = {}


def _build_nc():
    import concourse.bacc as bacc
    import concourse.tile as tile
    from concourse import mybir

    f32 = mybir.dt.float32
    nc = bacc.Bacc("TRN2", target_bir_lowering=False)
    yp_d = nc.dram_tensor("yp", [_P, _F], f32, kind="ExternalInput")
    wn_d = nc.dram_tensor("wn", [_P, _F], f32, kind="ExternalInput")
    wp_d = nc.dram_tensor("wp", [_P, _F], f32, kind="ExternalInput")
    out_d = nc.dram_tensor("out", [1, 1], f32, kind="ExternalOutput")

    with tile.TileContext(nc) as tc:
        with (
            tc.tile_pool(name="io", bufs=1) as io,
            tc.tile_pool(name="ps", bufs=1, space="PSUM") as ps,
        ):
            # three input loads on three different engine queues (parallel)
            t_yp = io.tile([_P, _F], f32)
            t_wn = io.tile([_P, _F], f32)
            t_wp = io.tile([_P, _F], f32)
            nc.sync.dma_start(out=t_yp[:, :], in_=yp_d[:, :])
            nc.scalar.dma_start(out=t_wn[:, :], in_=wn_d[:, :])
            nc.gpsimd.dma_start(out=t_wp[:, :], in_=wp_d[:, :])

            ones = io.tile([_P, 1], f32)
            nc.gpsimd.memset(ones[:, :], 1.0)

            # DVE chain; yp-only ops first so they overlap the wn/wp loads
            t_sq1 = io.tile([_P, _F], f32)
            nc.vector.tensor_mul(t_sq1[:, :], t_yp[:, :], t_yp[:, :])
            t_b = io.tile([_P, _F], f32)
            nc.vector.tensor_scalar_sub(t_b[:, :], t_yp[:, :], 20.0)
            t_sq2 = io.tile([_P, _F], f32)
            nc.vector.tensor_mul(t_sq2[:, :], t_b[:, :], t_b[:, :])
            t_m1 = io.tile([_P, _F], f32)
            nc.vector.tensor_mul(t_m1[:, :], t_sq1[:, :], t_wn[:, :])
            t_m2 = io.tile([_P, _F], f32)
            nc.vector.tensor_mul(t_m2[:, :], t_sq2[:, :], t_wp[:, :])

            # column sums via PE: psum[1,256] = ones.T @ m1 + ones.T @ m2
            col = ps.tile([1, _F], f32)
            nc.tensor.matmul(col[:, :], ones[:, :], t_m1[:, :], start=True, stop=False)
            nc.tensor.matmul(col[:, :], ones[:, :], t_m2[:, :], start=False, stop=True)

            # final reduce [1,256] -> [1,1] and 4-byte store
            t_out = io.tile([1, 1], f32)
            nc.vector.reduce_sum(
                out=t_out[:, :], in_=col[:, :], axis=mybir.AxisListType.X
            )
            nc.sync.dma_start(out=out_d[:, :], in_=t_out[:, :])
    nc.finalize()
    return nc


def _build_nc_raw3(surgery=False):
    """v3: single packed input X=[128,768] (yp | A=wn+wp | B=-40*wp),
    Horner form loss_e=(A*y+B)*y; the constant term 400*wp is corrected on
    host via -10*sum(B). Raw Bacc, manual semaphores."""
    from contextlib import ExitStack

    import concourse.bacc as bacc
    from concourse import mybir

    f32 = mybir.dt.float32
    nc = bacc.Bacc("TRN2", target_bir_lowering=False)
    x_d = nc.dram_tensor("x", [_P, 3 * _F], f32, kind="ExternalInput")
    out_d = nc.dram_tensor("out", [1, 1], f32, kind="ExternalOutput")

    with ExitStack() as ctx:
        t_x = ctx.enter_context(nc.sbuf_tensor([_P, 3 * _F], f32))
        t_1 = ctx.enter_context(nc.sbuf_tensor([_P, _F], f32))
        t_2 = ctx.enter_context(nc.sbuf_tensor([_P, _F], f32))
        t_3 = ctx.enter_context(nc.sbuf_tensor([_P, _F], f32))
        ones = ctx.enter_context(nc.sbuf_tensor([_P, 1], f32))
        p_t = ctx.enter_context(nc.sbuf_tensor([_P, 1], f32))
        t_out = ctx.enter_context(nc.sbuf_tensor([1, 1], f32))
        acc = ctx.enter_context(nc.psum_tensor([1, 1], f32))

        s_x = ctx.enter_context(nc.semaphore("s_x"))
        s_ones = ctx.enter_context(nc.semaphore("s_ones"))
        s_p = ctx.enter_context(nc.semaphore("s_p"))
        s_mm = ctx.enter_context(nc.semaphore("s_mm"))
        s_res = ctx.enter_context(nc.semaphore("s_res"))
        s_out = ctx.enter_context(nc.semaphore("s_out"))
        s_v = ctx.enter_context(nc.semaphore("s_v"))
        block = ctx.enter_context(nc.Block())

        yp = t_x[:, 0:_F]
        A = t_x[:, _F : 2 * _F]
        B = t_x[:, 2 * _F : 3 * _F]

        @block.sync
        def _(sync):
            sync.dma_start(out=t_x[:, :], in_=x_d[:, :]).then_inc(s_x, 16)
            sync.wait_ge(s_res, 1)
            sync.dma_start(out=out_d[:, :], in_=t_out[:, :]).then_inc(s_out, 16)
            sync.wait_ge(s_out, 16)

        @block.gpsimd
        def _(gpsimd):
            gpsimd.memset(ones[:, :], 1.0).then_inc(s_ones, 1)

        @block.vector
        def _(vector):
            n = [0]

            def step(ins):
                n[0] += 1
                ins.then_inc(s_v, 1)

            def fence():
                vector.wait_ge(s_v, n[0])

            vector.wait_ge(s_x, 16)
            step(nc.vector.tensor_mul(t_1[:, :], A, yp))
            fence()
            step(nc.vector.tensor_add(t_2[:, :], t_1[:, :], B))
            fence()
            step(nc.vector.tensor_mul(t_3[:, :], t_2[:, :], yp))
            fence()
            nc.vector.reduce_sum(
                out=p_t[:, :], in_=t_3[:, :], axis=mybir.AxisListType.X
            ).then_inc(s_p, 1)
            vector.wait_ge(s_mm, 1)
            nc.vector.tensor_copy(t_out[:, :], acc[:, :]).then_inc(s_res, 1)

        @block.tensor
        def _(tensor):
            tensor.wait_ge(s_p, 1)
            tensor.wait_ge(s_ones, 1)
            nc.tensor.matmul(
                acc[:, :], p_t[:, :], ones[:, :], start=True, stop=True
            ).then_inc(s_mm, 1)

    if surgery:
        _strip_barriers(nc)
    nc.finalize()
    return nc


def _strip_barriers(nc):
    """Remove the main-block entry barrier round, the unused const-AP
    memsets, and the Block-end barrier round. Only touches the framework's
    prologue/epilogue blocks; cross-engine deps in the engine blocks are
    fully covered by explicit semaphores."""
    from concourse import mybir

    for bb in nc.main_func.blocks:
        if bb.name != "main" and not bb.name.endswith("_end"):
            continue
        keep = []
        for ins in bb.instructions:
            if isinstance(ins, (mybir.InstDrain, mybir.InstEventSemaphore)):
                continue
            if bb.name == "main" and isinstance(ins, mybir.InstMemset):
                outs = getattr(ins, "outs", [])
                names = str(outs)
                if "const-" in names:
                    continue
            keep.append(ins)
        bb.instructions[:] = keep


def _build_nc_raw():
    """Raw Bacc kernel with manual semaphores — skips the Tile framework's
    entry/exit all-engine barriers, ordering modes, and extra prologue."""
    from contextlib import ExitStack

    import concourse.bacc as bacc
    from concourse import mybir

    f32 = mybir.dt.float32
    nc = bacc.Bacc("TRN2", target_bir_lowering=False)
    yp_d = nc.dram_tensor("yp", [_P, _F], f32, kind="ExternalInput")
    wn_d = nc.dram_tensor("wn", [_P, _F], f32, kind="ExternalInput")
    wp_d = nc.dram_tensor("wp", [_P, _F], f32, kind="ExternalInput")
    out_d = nc.dram_tensor("out", [1, 1], f32, kind="ExternalOutput")

    with ExitStack() as ctx:
        t_yp = ctx.enter_context(nc.sbuf_tensor([_P, _F], f32))
        t_wn = ctx.enter_context(nc.sbuf_tensor([_P, _F], f32))
        t_wp = ctx.enter_context(nc.sbuf_tensor([_P, _F], f32))
        t_sq1 = ctx.enter_context(nc.sbuf_tensor([_P, _F], f32))
        t_b = ctx.enter_context(nc.sbuf_tensor([_P, _F], f32))
        t_sq2 = ctx.enter_context(nc.sbuf_tensor([_P, _F], f32))
        t_m1 = ctx.enter_context(nc.sbuf_tensor([_P, _F], f32))
        t_m2 = ctx.enter_context(nc.sbuf_tensor([_P, _F], f32))
        ones = ctx.enter_context(nc.sbuf_tensor([_P, 1], f32))
        p_a = ctx.enter_context(nc.sbuf_tensor([_P, 1], f32))
        p_b = ctx.enter_context(nc.sbuf_tensor([_P, 1], f32))
        p_t = ctx.enter_context(nc.sbuf_tensor([_P, 1], f32))
        t_out = ctx.enter_context(nc.sbuf_tensor([1, 1], f32))
        acc = ctx.enter_context(nc.psum_tensor([1, 1], f32))

        s_yp = ctx.enter_context(nc.semaphore("s_yp"))
        s_wn = ctx.enter_context(nc.semaphore("s_wn"))
        s_wp = ctx.enter_context(nc.semaphore("s_wp"))
        s_ones = ctx.enter_context(nc.semaphore("s_ones"))
        s_p = ctx.enter_context(nc.semaphore("s_p"))
        s_mm = ctx.enter_context(nc.semaphore("s_mm"))
        s_res = ctx.enter_context(nc.semaphore("s_res"))
        s_out = ctx.enter_context(nc.semaphore("s_out"))
        s_v = ctx.enter_context(nc.semaphore("s_v"))
        block = ctx.enter_context(nc.Block())

        @block.sync
        def _(sync):
            sync.dma_start(out=t_yp[:, :], in_=yp_d[:, :]).then_inc(s_yp, 16)
            sync.wait_ge(s_res, 1)
            sync.dma_start(out=out_d[:, :], in_=t_out[:, :]).then_inc(s_out, 16)
            sync.wait_ge(s_out, 16)

        @block.scalar
        def _(scalar):
            scalar.dma_start(out=t_wn[:, :], in_=wn_d[:, :]).then_inc(s_wn, 16)

        @block.gpsimd
        def _(gpsimd):
            gpsimd.memset(ones[:, :], 1.0).then_inc(s_ones, 1)
            gpsimd.dma_start(out=t_wp[:, :], in_=wp_d[:, :]).then_inc(s_wp, 16)

        @block.vector
        def _(vector):
            # s_v serializes the DVE RAW chains (the engine pipeline does
            # not interlock same-engine SBUF read-after-write).
            n = [0]

            def step(ins):
                n[0] += 1
                ins.then_inc(s_v, 1)

            def fence():
                vector.wait_ge(s_v, n[0])

            vector.wait_ge(s_yp, 16)
            step(nc.vector.tensor_mul(t_sq1[:, :], t_yp[:, :], t_yp[:, :]))
            step(nc.vector.tensor_scalar_sub(t_b[:, :], t_yp[:, :], 20.0))
            fence()
            step(nc.vector.tensor_mul(t_sq2[:, :], t_b[:, :], t_b[:, :]))
            vector.wait_ge(s_wn, 16)
            fence()
            step(nc.vector.tensor_mul(t_m1[:, :], t_sq1[:, :], t_wn[:, :]))
            fence()
            step(
                nc.vector.reduce_sum(
                    out=p_a[:, :], in_=t_m1[:, :], axis=mybir.AxisListType.X
                )
            )
            vector.wait_ge(s_wp, 16)
            step(nc.vector.tensor_mul(t_m2[:, :], t_sq2[:, :], t_wp[:, :]))
            fence()
            step(
                nc.vector.reduce_sum(
                    out=p_b[:, :], in_=t_m2[:, :], axis=mybir.AxisListType.X
                )
            )
            fence()
            nc.vector.tensor_add(p_t[:, :], p_a[:, :], p_b[:, :]).then_inc(s_p, 1)
            vector.wait_ge(s_mm, 1)
            nc.vector.tensor_copy(t_out[:, :], acc[:, :]).then_inc(s_res, 1)

        @block.tensor
        def _(tensor):
            tensor.wait_ge(s_p, 1)
            tensor.wait_ge(s_ones, 1)
            nc.tensor.matmul(
                acc[:, :], p_t[:, :], ones[:, :], start=True, stop=True
            ).then_inc(s_mm, 1)

    nc.finalize()
    return nc


def _impl():
    import os

    return os.environ.get("K_IMPL", "raw3s")


def _get_nc():
    if "nc" not in _NC_CACHE:
        impl = _impl()
        if impl == "tile":
            _NC_CACHE["nc"] = _build_nc()
        elif impl == "raw":
            _NC_CACHE["nc"] = _build_nc_raw()
        elif impl == "raw3":
            _NC_CACHE["nc"] = _build_nc_raw3(surgery=False)
        else:
            _NC_CACHE["nc"] = _build_nc_raw3(surgery=True)
    return _NC_CACHE["nc"]


def _shard(arr):
    """(4,1,256,256) f32 -> list of 8 [128, 256] per-core chunks."""
    flat = np.ascontiguousarray(arr, dtype=np.float32).reshape(N_CORES, _P, _F)
    return [flat[c] for c in range(N_CORES)]


def run_device(y_pred, wn_img, wp_img, trace=False, **kw):
    from concourse.bass_utils import run_bass_kernel_spmd

    nc = _get_nc()
    impl = _impl()
    if impl in ("tile", "raw"):
        yps = _shard(y_pred)
        wns = _shard(wn_img)
        wps = _shard(wp_img)
        in_maps = [
            {"yp": yps[c], "wn": wns[c], "wp": wps[c]} for c in range(N_CORES)
        ]
        correction = 0.0
    else:
        A = wn_img + wp_img
        B = wp_img * np.float32(-40.0)
        xs = np.concatenate(
            [
                np.ascontiguousarray(y_pred, dtype=np.float32).reshape(
                    N_CORES, _P, _F
                ),
                A.reshape(N_CORES, _P, _F),
                B.reshape(N_CORES, _P, _F),
            ],
            axis=2,
        )
        in_maps = [{"x": np.ascontiguousarray(xs[c])} for c in range(N_CORES)]
        # device returns sum((A*y+B)*y); the constant term 400*wp == -10*B
        correction = -10.0 * B.astype(np.float64).sum()
    res = run_bass_kernel_spmd(
        nc, in_maps, core_ids=list(range(N_CORES)), trace=trace, **kw
    )
    partials = np.array(
        [float(res.results[c]["out"][0, 0]) for c in range(N_CORES)], dtype=np.float64
    )
    total = np.float32(partials.sum() + correction)
    return total, res


def kernel(y_true, y_pred):
    y_true = np.asarray(y_true, dtype=np.float32)
    y_pred = np.asarray(y_pred, dtype=np.float32)
    wn_img, wp_img = malis_weights_full(y_pred, y_true)
    total, _ = run_device(y_pred, wn_img, wp_img, trace=False)
    return np.array(total, dtype=np.float32)


# revision 18
# speedup vs baseline: 1.6219x; 1.0138x over previous
"""Trainium2 kernel for nn_ConnectivityLoss (MALIS connectivity loss).

Contract: kernel(y_true, y_pred) -> scalar float32 loss, matching

    loss = sum(y_pred**2 * weights_n + (20 - y_pred)**2 * weights_p)

where weights_n / weights_p are the MALIS maximin edge weights computed per
32x32 window.  The reference itself computes the weights with a host
``jax.pure_callback`` (sequential Kruskal + union-find is not device work);
this kernel mirrors that split: a fast bit-exact host implementation of the
weights, and the memory-bound weighted reduction on 8 NeuronCores
(data-parallel over the flattened pixels).
"""

import numpy as np

# ===========================================================================
# Host side: bit-exact fast reimplementation of the reference MALIS weights.
# ===========================================================================

WIN = 32
_N = WIN * WIN
_idx = np.arange(_N).reshape(WIN, WIN)
E1 = np.concatenate([_idx[:, :-1].ravel(), _idx[:-1, :].ravel()]).astype(np.int64)
E2 = np.concatenate([_idx[:, 1:].ravel(), _idx[1:, :].ravel()]).astype(np.int64)
_E = E1.shape[0]  # 1984
_HALF = WIN * (WIN - 1)  # 992

try:
    from scipy import ndimage as _ndi

    def _label_bg(bg):  # bg: bool (WIN, WIN) -> int32 labels, 0 = unlabeled
        lab, _ = _ndi.label(bg)
        return lab.astype(np.int32)
except Exception:  # pragma: no cover

    def _label_bg(bg):
        lab = np.zeros((WIN, WIN), np.int32)
        nxt = 0
        stack = []
        for i in range(WIN):
            for j in range(WIN):
                if bg[i, j] and lab[i, j] == 0:
                    nxt += 1
                    stack.append((i, j))
                    lab[i, j] = nxt
                    while stack:
                        a, b = stack.pop()
                        for u, v in ((a - 1, b), (a + 1, b), (a, b - 1), (a, b + 1)):
                            if 0 <= u < WIN and 0 <= v < WIN and bg[u, v] and lab[u, v] == 0:
                                lab[u, v] = nxt
                                stack.append((u, v))
        return lab


def _malis_runs_py(orders, segs, e1, e2, pos):
    """Pure-python fallback: Kruskal maximin MALIS weighting, R runs."""
    R, E = orders.shape
    N = segs.shape[1]
    w = np.zeros((R, E), np.int64)
    for r in range(R):
        seg = segs[r]
        parent = np.arange(N, dtype=np.int64)
        cnts = [({int(seg[i]): 1} if seg[i] != 0 else {}) for i in range(N)]
        tot = [len(c) for c in cnts]
        wr = w[r]
        order = orders[r]
        for k in range(E):
            e = order[k]
            a = e1[e]
            while parent[a] != a:
                parent[a] = parent[parent[a]]
                a = parent[a]
            b = e2[e]
            while parent[b] != b:
                parent[b] = parent[parent[b]]
                b = parent[b]
            if a == b:
                continue
            ca, cb = cnts[a], cnts[b]
            if len(ca) > len(cb):
                a, b, ca, cb = b, a, cb, ca
            common = 0
            for l, c in ca.items():
                if l in cb:
                    common += c * cb[l]
            if pos:
                wr[e] = common
            else:
                wr[e] = tot[a] * tot[b] - common
            for l, c in ca.items():
                cb[l] = cb.get(l, 0) + c
            tot[b] += tot[a]
            parent[a] = b
            cnts[a] = {}
    return w


_malis_runs = None
try:
    import numba

    @numba.njit(cache=True)
    def _malis_runs_nb(orders, segs, e1, e2, pos):
        R, E = orders.shape
        N = segs.shape[1]
        w = np.zeros((R, E), np.int64)
        for r in range(R):
            seg = segs[r]
            L = 0
            for i in range(N):
                if seg[i] > L:
                    L = seg[i]
            parent = np.arange(N, dtype=np.int64)
            cnt = np.zeros((N, L + 1), np.int64)
            tot = np.zeros(N, np.int64)
            for i in range(N):
                if seg[i] != 0:
                    cnt[i, seg[i]] = 1
                    tot[i] = 1
            for k in range(E):
                e = orders[r, k]
                a = e1[e]
                while parent[a] != a:
                    parent[a] = parent[parent[a]]
                    a = parent[a]
                b = e2[e]
                while parent[b] != b:
                    parent[b] = parent[parent[b]]
                    b = parent[b]
                if a == b:
                    continue
                common = np.int64(0)
                for l in range(1, L + 1):
                    common += cnt[a, l] * cnt[b, l]
                if pos:
                    w[r, e] = common
                else:
                    w[r, e] = tot[a] * tot[b] - common
                for l in range(1, L + 1):
                    cnt[b, l] += cnt[a, l]
                tot[b] += tot[a]
                parent[a] = b
        return w

    _malis_runs = _malis_runs_nb
except Exception:  # pragma: no cover
    _malis_runs = None


def malis_weights_full(pred, target):
    """Bit-exact clone of the reference's _malis_weights_full."""
    pred = np.ascontiguousarray(np.asarray(pred, dtype=np.float32))
    target = np.ascontiguousarray(np.asarray(target, dtype=np.float32))
    B, C, H, W = pred.shape
    nR, nC = H // WIN, W // WIN
    R = B * nR * nC

    p = pred[:, 0].reshape(B, nR, WIN, nC, WIN).transpose(0, 1, 3, 2, 4)
    t = target[:, 0].reshape(B, nR, WIN, nC, WIN).transpose(0, 1, 3, 2, 4)

    costs_h = (p[..., :, :-1] + p[..., :, 1:]).reshape(B, nR, nC, _HALF)
    costs_v = (p[..., :-1, :] + p[..., 1:, :]).reshape(B, nR, nC, _HALF)
    costs = np.concatenate([costs_h, costs_v], axis=-1)  # (B,nR,nC,E) f32
    gt_h = (t[..., :, :-1] + t[..., :, 1:]).reshape(B, nR, nC, _HALF)
    gt_v = (t[..., :-1, :] + t[..., 1:, :]).reshape(B, nR, nC, _HALF)
    gt = np.concatenate([gt_h, gt_v], axis=-1)

    costs_n = costs.copy()
    costs_p = costs.copy()
    costs_n[gt > 20] = 20
    costs_p[gt < 10] = 0
    gtc = np.minimum(gt, 20)

    # stable descending argsort — identical tie-breaking to the reference
    order_n = np.ascontiguousarray(
        np.argsort(-costs_n, axis=-1, kind="stable").reshape(R, _E)
    )
    order_p = np.ascontiguousarray(
        np.argsort(-costs_p, axis=-1, kind="stable").reshape(R, _E)
    )

    segs = np.empty((B, nR, nC, _N), np.int32)
    bg = t == 0.0
    for b in range(B):
        for r in range(nR):
            for c in range(nC):
                segs[b, r, c] = _label_bg(bg[b, r, c]).ravel()
    segs2 = segs.reshape(R, _N)

    global _malis_runs
    if _malis_runs is not None:
        try:
            wn = _malis_runs(order_n, segs2, E1, E2, 0)
            wp = _malis_runs(order_p, segs2, E1, E2, 1)
        except Exception:
            _malis_runs = None
            wn = _malis_runs_py(order_n, segs2, E1, E2, 0)
            wp = _malis_runs_py(order_p, segs2, E1, E2, 1)
    else:
        wn = _malis_runs_py(order_n, segs2, E1, E2, 0)
        wp = _malis_runs_py(order_p, segs2, E1, E2, 1)

    out = []
    gtc_flat = gtc.reshape(R, _E)
    for w, is_pos in ((wn, False), (wp, True)):
        w64 = w.astype(np.float64)
        s = w64.sum(axis=-1, keepdims=True)
        np.divide(w64, s, out=w64, where=s > 0)
        if is_pos:
            w64[gtc_flat < 20] = 0
        else:
            w64[gtc_flat >= 10] = 0
        wh = w64[:, :_HALF].reshape(R, WIN, WIN - 1)
        wv = w64[:, _HALF:].reshape(R, WIN - 1, WIN)
        nw = np.zeros((R, WIN, WIN), np.float64)
        nw[:, :, :-1] += wh
        nw[:, :, 1:] += wh
        nw[:, :-1, :] += wv
        nw[:, 1:, :] += wv
        img = (
            nw.reshape(B, nR, nC, WIN, WIN)
            .transpose(0, 1, 3, 2, 4)
            .reshape(B, 1, H, W)
            .astype(np.float32)
        )
        out.append(img)
    return out[0], out[1]


# ===========================================================================
# Device side: weighted-loss reduction on 8 NeuronCores.
# ===========================================================================

N_CORES = 8
_P = 128  # SBUF partitions
_TOT = 4 * 1 * 256 * 256  # 262144 pixels
_PER_CORE = _TOT // N_CORES  # 32768
_F = _PER_CORE // _P  # 256 floats per partition per tensor

_NC_CACHE = {}


def _build_nc():
    import concourse.bacc as bacc
    import concourse.tile as tile
    from concourse import mybir

    f32 = mybir.dt.float32
    nc = bacc.Bacc("TRN2", target_bir_lowering=False)
    yp_d = nc.dram_tensor("yp", [_P, _F], f32, kind="ExternalInput")
    wn_d = nc.dram_tensor("wn", [_P, _F], f32, kind="ExternalInput")
    wp_d = nc.dram_tensor("wp", [_P, _F], f32, kind="ExternalInput")
    out_d = nc.dram_tensor("out", [1, 1], f32, kind="ExternalOutput")

    with tile.TileContext(nc) as tc:
        with (
            tc.tile_pool(name="io", bufs=1) as io,
            tc.tile_pool(name="ps", bufs=1, space="PSUM") as ps,
        ):
            # three input loads on three different engine queues (parallel)
            t_yp = io.tile([_P, _F], f32)
            t_wn = io.tile([_P, _F], f32)
            t_wp = io.tile([_P, _F], f32)
            nc.sync.dma_start(out=t_yp[:, :], in_=yp_d[:, :])
            nc.scalar.dma_start(out=t_wn[:, :], in_=wn_d[:, :])
            nc.gpsimd.dma_start(out=t_wp[:, :], in_=wp_d[:, :])

            ones = io.tile([_P, 1], f32)
            nc.gpsimd.memset(ones[:, :], 1.0)

            # DVE chain; yp-only ops first so they overlap the wn/wp loads
            t_sq1 = io.tile([_P, _F], f32)
            nc.vector.tensor_mul(t_sq1[:, :], t_yp[:, :], t_yp[:, :])
            t_b = io.tile([_P, _F], f32)
            nc.vector.tensor_scalar_sub(t_b[:, :], t_yp[:, :], 20.0)
            t_sq2 = io.tile([_P, _F], f32)
            nc.vector.tensor_mul(t_sq2[:, :], t_b[:, :], t_b[:, :])
            t_m1 = io.tile([_P, _F], f32)
            nc.vector.tensor_mul(t_m1[:, :], t_sq1[:, :], t_wn[:, :])
            t_m2 = io.tile([_P, _F], f32)
            nc.vector.tensor_mul(t_m2[:, :], t_sq2[:, :], t_wp[:, :])

            # column sums via PE: psum[1,256] = ones.T @ m1 + ones.T @ m2
            col = ps.tile([1, _F], f32)
            nc.tensor.matmul(col[:, :], ones[:, :], t_m1[:, :], start=True, stop=False)
            nc.tensor.matmul(col[:, :], ones[:, :], t_m2[:, :], start=False, stop=True)

            # final reduce [1,256] -> [1,1] and 4-byte store
            t_out = io.tile([1, 1], f32)
            nc.vector.reduce_sum(
                out=t_out[:, :], in_=col[:, :], axis=mybir.AxisListType.X
            )
            nc.sync.dma_start(out=out_d[:, :], in_=t_out[:, :])
    nc.finalize()
    return nc


_DROP_OUT_WAIT = False


def _build_nc_raw3(surgery=False):
    """v3: single packed input X=[128,768] (yp | A=wn+wp | B=-40*wp) loaded
    as two partition-halves on the two HWDGE queues (sync+scalar),
    Horner form loss_e=(A*y+B)*y; the constant term 400*wp is corrected on
    host via -10*sum(B). Raw Bacc, manual semaphores."""
    from contextlib import ExitStack

    import concourse.bacc as bacc
    from concourse import mybir

    f32 = mybir.dt.float32
    nc = bacc.Bacc("TRN2", target_bir_lowering=False)
    x_d = nc.dram_tensor("x", [_P, 3 * _F], f32, kind="ExternalInput")
    out_d = nc.dram_tensor("out", [1, 1], f32, kind="ExternalOutput")

    with ExitStack() as ctx:
        t_x = ctx.enter_context(nc.sbuf_tensor([_P, 3 * _F], f32))
        t_1 = ctx.enter_context(nc.sbuf_tensor([_P, _F], f32))
        t_2 = ctx.enter_context(nc.sbuf_tensor([_P, _F], f32))
        t_3 = ctx.enter_context(nc.sbuf_tensor([_P, _F], f32))
        ones = ctx.enter_context(nc.sbuf_tensor([_P, 1], f32))
        p_t = ctx.enter_context(nc.sbuf_tensor([_P, 1], f32))
        t_out = ctx.enter_context(nc.sbuf_tensor([1, 1], f32))
        acc = ctx.enter_context(nc.psum_tensor([1, 1], f32))

        s_x = ctx.enter_context(nc.semaphore("s_x"))
        s_ones = ctx.enter_context(nc.semaphore("s_ones"))
        s_p = ctx.enter_context(nc.semaphore("s_p"))
        s_mm = ctx.enter_context(nc.semaphore("s_mm"))
        s_res = ctx.enter_context(nc.semaphore("s_res"))
        s_out = ctx.enter_context(nc.semaphore("s_out"))
        s_v = ctx.enter_context(nc.semaphore("s_v"))
        block = ctx.enter_context(nc.Block())

        yp = t_x[:, 0:_F]
        A = t_x[:, _F : 2 * _F]
        B = t_x[:, 2 * _F : 3 * _F]
        H = _P // 2

        @block.sync
        def _(sync):
            sync.dma_start(out=t_x[:H, :], in_=x_d[:H, :]).then_inc(s_x, 16)
            sync.wait_ge(s_res, 1)
            sync.dma_start(out=out_d[:, :], in_=t_out[:, :]).then_inc(s_out, 16)
            if not _DROP_OUT_WAIT:
                sync.wait_ge(s_out, 16)

        @block.scalar
        def _(scalar):
            scalar.dma_start(out=t_x[H:, :], in_=x_d[H:, :]).then_inc(s_x, 16)

        @block.gpsimd
        def _(gpsimd):
            gpsimd.memset(ones[:, :], 1.0).then_inc(s_ones, 1)

        @block.vector
        def _(vector):
            n = [0]

            def step(ins):
                n[0] += 1
                ins.then_inc(s_v, 1)

            def fence():
                vector.wait_ge(s_v, n[0])

            vector.wait_ge(s_x, 32)
            step(nc.vector.tensor_mul(t_1[:, :], A, yp))
            fence()
            step(nc.vector.tensor_add(t_2[:, :], t_1[:, :], B))
            fence()
            step(nc.vector.tensor_mul(t_3[:, :], t_2[:, :], yp))
            fence()
            nc.vector.reduce_sum(
                out=p_t[:, :], in_=t_3[:, :], axis=mybir.AxisListType.X
            ).then_inc(s_p, 1)
            vector.wait_ge(s_mm, 1)
            nc.vector.tensor_copy(t_out[:, :], acc[:, :]).then_inc(s_res, 1)

        @block.tensor
        def _(tensor):
            tensor.wait_ge(s_p, 1)
            tensor.wait_ge(s_ones, 1)
            nc.tensor.matmul(
                acc[:, :], p_t[:, :], ones[:, :], start=True, stop=True
            ).then_inc(s_mm, 1)

    if surgery:
        _strip_barriers(nc)
    nc.finalize()
    return nc


def _strip_barriers(nc):
    """Remove the main-block entry barrier round, the unused const-AP
    memsets, and the Block-end barrier round. Only touches the framework's
    prologue/epilogue blocks; cross-engine deps in the engine blocks are
    fully covered by explicit semaphores."""
    from concourse import mybir

    for bb in nc.main_func.blocks:
        if bb.name != "main" and not bb.name.endswith("_end"):
            continue
        keep = []
        for ins in bb.instructions:
            if isinstance(ins, (mybir.InstDrain, mybir.InstEventSemaphore)):
                continue
            if bb.name == "main" and isinstance(ins, mybir.InstMemset):
                outs = getattr(ins, "outs", [])
                names = str(outs)
                if "const-" in names:
                    continue
            keep.append(ins)
        bb.instructions[:] = keep


def _build_nc_raw():
    """Raw Bacc kernel with manual semaphores — skips the Tile framework's
    entry/exit all-engine barriers, ordering modes, and extra prologue."""
    from contextlib import ExitStack

    import concourse.bacc as bacc
    from concourse import mybir

    f32 = mybir.dt.float32
    nc = bacc.Bacc("TRN2", target_bir_lowering=False)
    yp_d = nc.dram_tensor("yp", [_P, _F], f32, kind="ExternalInput")
    wn_d = nc.dram_tensor("wn", [_P, _F], f32, kind="ExternalInput")
    wp_d = nc.dram_tensor("wp", [_P, _F], f32, kind="ExternalInput")
    out_d = nc.dram_tensor("out", [1, 1], f32, kind="ExternalOutput")

    with ExitStack() as ctx:
        t_yp = ctx.enter_context(nc.sbuf_tensor([_P, _F], f32))
        t_wn = ctx.enter_context(nc.sbuf_tensor([_P, _F], f32))
        t_wp = ctx.enter_context(nc.sbuf_tensor([_P, _F], f32))
        t_sq1 = ctx.enter_context(nc.sbuf_tensor([_P, _F], f32))
        t_b = ctx.enter_context(nc.sbuf_tensor([_P, _F], f32))
        t_sq2 = ctx.enter_context(nc.sbuf_tensor([_P, _F], f32))
        t_m1 = ctx.enter_context(nc.sbuf_tensor([_P, _F], f32))
        t_m2 = ctx.enter_context(nc.sbuf_tensor([_P, _F], f32))
        ones = ctx.enter_context(nc.sbuf_tensor([_P, 1], f32))
        p_a = ctx.enter_context(nc.sbuf_tensor([_P, 1], f32))
        p_b = ctx.enter_context(nc.sbuf_tensor([_P, 1], f32))
        p_t = ctx.enter_context(nc.sbuf_tensor([_P, 1], f32))
        t_out = ctx.enter_context(nc.sbuf_tensor([1, 1], f32))
        acc = ctx.enter_context(nc.psum_tensor([1, 1], f32))

        s_yp = ctx.enter_context(nc.semaphore("s_yp"))
        s_wn = ctx.enter_context(nc.semaphore("s_wn"))
        s_wp = ctx.enter_context(nc.semaphore("s_wp"))
        s_ones = ctx.enter_context(nc.semaphore("s_ones"))
        s_p = ctx.enter_context(nc.semaphore("s_p"))
        s_mm = ctx.enter_context(nc.semaphore("s_mm"))
        s_res = ctx.enter_context(nc.semaphore("s_res"))
        s_out = ctx.enter_context(nc.semaphore("s_out"))
        s_v = ctx.enter_context(nc.semaphore("s_v"))
        block = ctx.enter_context(nc.Block())

        @block.sync
        def _(sync):
            sync.dma_start(out=t_yp[:, :], in_=yp_d[:, :]).then_inc(s_yp, 16)
            sync.wait_ge(s_res, 1)
            sync.dma_start(out=out_d[:, :], in_=t_out[:, :]).then_inc(s_out, 16)
            sync.wait_ge(s_out, 16)

        @block.scalar
        def _(scalar):
            scalar.dma_start(out=t_wn[:, :], in_=wn_d[:, :]).then_inc(s_wn, 16)

        @block.gpsimd
        def _(gpsimd):
            gpsimd.memset(ones[:, :], 1.0).then_inc(s_ones, 1)
            gpsimd.dma_start(out=t_wp[:, :], in_=wp_d[:, :]).then_inc(s_wp, 16)

        @block.vector
        def _(vector):
            # s_v serializes the DVE RAW chains (the engine pipeline does
            # not interlock same-engine SBUF read-after-write).
            n = [0]

            def step(ins):
                n[0] += 1
                ins.then_inc(s_v, 1)

            def fence():
                vector.wait_ge(s_v, n[0])

            vector.wait_ge(s_yp, 16)
            step(nc.vector.tensor_mul(t_sq1[:, :], t_yp[:, :], t_yp[:, :]))
            step(nc.vector.tensor_scalar_sub(t_b[:, :], t_yp[:, :], 20.0))
            fence()
            step(nc.vector.tensor_mul(t_sq2[:, :], t_b[:, :], t_b[:, :]))
            vector.wait_ge(s_wn, 16)
            fence()
            step(nc.vector.tensor_mul(t_m1[:, :], t_sq1[:, :], t_wn[:, :]))
            fence()
            step(
                nc.vector.reduce_sum(
                    out=p_a[:, :], in_=t_m1[:, :], axis=mybir.AxisListType.X
                )
            )
            vector.wait_ge(s_wp, 16)
            step(nc.vector.tensor_mul(t_m2[:, :], t_sq2[:, :], t_wp[:, :]))
            fence()
            step(
                nc.vector.reduce_sum(
                    out=p_b[:, :], in_=t_m2[:, :], axis=mybir.AxisListType.X
                )
            )
            fence()
            nc.vector.tensor_add(p_t[:, :], p_a[:, :], p_b[:, :]).then_inc(s_p, 1)
            vector.wait_ge(s_mm, 1)
            nc.vector.tensor_copy(t_out[:, :], acc[:, :]).then_inc(s_res, 1)

        @block.tensor
        def _(tensor):
            tensor.wait_ge(s_p, 1)
            tensor.wait_ge(s_ones, 1)
            nc.tensor.matmul(
                acc[:, :], p_t[:, :], ones[:, :], start=True, stop=True
            ).then_inc(s_mm, 1)

    nc.finalize()
    return nc


def _impl():
    import os

    return os.environ.get("K_IMPL", "raw3s")


def _get_nc():
    if "nc" not in _NC_CACHE:
        impl = _impl()
        if impl == "tile":
            _NC_CACHE["nc"] = _build_nc()
        elif impl == "raw":
            _NC_CACHE["nc"] = _build_nc_raw()
        elif impl == "raw3":
            _NC_CACHE["nc"] = _build_nc_raw3(surgery=False)
        else:
            _NC_CACHE["nc"] = _build_nc_raw3(surgery=True)
    return _NC_CACHE["nc"]


def _shard(arr):
    """(4,1,256,256) f32 -> list of 8 [128, 256] per-core chunks."""
    flat = np.ascontiguousarray(arr, dtype=np.float32).reshape(N_CORES, _P, _F)
    return [flat[c] for c in range(N_CORES)]


def run_device(y_pred, wn_img, wp_img, trace=False, **kw):
    from concourse.bass_utils import run_bass_kernel_spmd

    nc = _get_nc()
    impl = _impl()
    if impl in ("tile", "raw"):
        yps = _shard(y_pred)
        wns = _shard(wn_img)
        wps = _shard(wp_img)
        in_maps = [
            {"yp": yps[c], "wn": wns[c], "wp": wps[c]} for c in range(N_CORES)
        ]
        correction = 0.0
    else:
        A = wn_img + wp_img
        B = wp_img * np.float32(-40.0)
        xs = np.concatenate(
            [
                np.ascontiguousarray(y_pred, dtype=np.float32).reshape(
                    N_CORES, _P, _F
                ),
                A.reshape(N_CORES, _P, _F),
                B.reshape(N_CORES, _P, _F),
            ],
            axis=2,
        )
        in_maps = [{"x": np.ascontiguousarray(xs[c])} for c in range(N_CORES)]
        # device returns sum((A*y+B)*y); the constant term 400*wp == -10*B
        correction = -10.0 * B.astype(np.float64).sum()
    res = run_bass_kernel_spmd(
        nc, in_maps, core_ids=list(range(N_CORES)), trace=trace, **kw
    )
    partials = np.array(
        [float(res.results[c]["out"][0, 0]) for c in range(N_CORES)], dtype=np.float64
    )
    total = np.float32(partials.sum() + correction)
    return total, res


def kernel(y_true, y_pred):
    y_true = np.asarray(y_true, dtype=np.float32)
    y_pred = np.asarray(y_pred, dtype=np.float32)
    wn_img, wp_img = malis_weights_full(y_pred, y_true)
    total, _ = run_device(y_pred, wn_img, wp_img, trace=False)
    return np.array(total, dtype=np.float32)


# revision 20
# speedup vs baseline: 1.7262x; 1.0643x over previous
"""Trainium2 kernel for nn_ConnectivityLoss (MALIS connectivity loss).

Contract: kernel(y_true, y_pred) -> scalar float32 loss, matching

    loss = sum(y_pred**2 * weights_n + (20 - y_pred)**2 * weights_p)

where weights_n / weights_p are the MALIS maximin edge weights computed per
32x32 window.  The reference itself computes the weights with a host
``jax.pure_callback`` (sequential Kruskal + union-find is not device work);
this kernel mirrors that split: a fast bit-exact host implementation of the
weights, and the memory-bound weighted reduction on 8 NeuronCores
(data-parallel over the flattened pixels).
"""

import numpy as np

# ===========================================================================
# Host side: bit-exact fast reimplementation of the reference MALIS weights.
# ===========================================================================

WIN = 32
_N = WIN * WIN
_idx = np.arange(_N).reshape(WIN, WIN)
E1 = np.concatenate([_idx[:, :-1].ravel(), _idx[:-1, :].ravel()]).astype(np.int64)
E2 = np.concatenate([_idx[:, 1:].ravel(), _idx[1:, :].ravel()]).astype(np.int64)
_E = E1.shape[0]  # 1984
_HALF = WIN * (WIN - 1)  # 992

try:
    from scipy import ndimage as _ndi

    def _label_bg(bg):  # bg: bool (WIN, WIN) -> int32 labels, 0 = unlabeled
        lab, _ = _ndi.label(bg)
        return lab.astype(np.int32)
except Exception:  # pragma: no cover

    def _label_bg(bg):
        lab = np.zeros((WIN, WIN), np.int32)
        nxt = 0
        stack = []
        for i in range(WIN):
            for j in range(WIN):
                if bg[i, j] and lab[i, j] == 0:
                    nxt += 1
                    stack.append((i, j))
                    lab[i, j] = nxt
                    while stack:
                        a, b = stack.pop()
                        for u, v in ((a - 1, b), (a + 1, b), (a, b - 1), (a, b + 1)):
                            if 0 <= u < WIN and 0 <= v < WIN and bg[u, v] and lab[u, v] == 0:
                                lab[u, v] = nxt
                                stack.append((u, v))
        return lab


def _malis_runs_py(orders, segs, e1, e2, pos):
    """Pure-python fallback: Kruskal maximin MALIS weighting, R runs."""
    R, E = orders.shape
    N = segs.shape[1]
    w = np.zeros((R, E), np.int64)
    for r in range(R):
        seg = segs[r]
        parent = np.arange(N, dtype=np.int64)
        cnts = [({int(seg[i]): 1} if seg[i] != 0 else {}) for i in range(N)]
        tot = [len(c) for c in cnts]
        wr = w[r]
        order = orders[r]
        for k in range(E):
            e = order[k]
            a = e1[e]
            while parent[a] != a:
                parent[a] = parent[parent[a]]
                a = parent[a]
            b = e2[e]
            while parent[b] != b:
                parent[b] = parent[parent[b]]
                b = parent[b]
            if a == b:
                continue
            ca, cb = cnts[a], cnts[b]
            if len(ca) > len(cb):
                a, b, ca, cb = b, a, cb, ca
            common = 0
            for l, c in ca.items():
                if l in cb:
                    common += c * cb[l]
            if pos:
                wr[e] = common
            else:
                wr[e] = tot[a] * tot[b] - common
            for l, c in ca.items():
                cb[l] = cb.get(l, 0) + c
            tot[b] += tot[a]
            parent[a] = b
            cnts[a] = {}
    return w


_malis_runs = None
try:
    import numba

    @numba.njit(cache=True)
    def _malis_runs_nb(orders, segs, e1, e2, pos):
        R, E = orders.shape
        N = segs.shape[1]
        w = np.zeros((R, E), np.int64)
        for r in range(R):
            seg = segs[r]
            L = 0
            for i in range(N):
                if seg[i] > L:
                    L = seg[i]
            parent = np.arange(N, dtype=np.int64)
            cnt = np.zeros((N, L + 1), np.int64)
            tot = np.zeros(N, np.int64)
            for i in range(N):
                if seg[i] != 0:
                    cnt[i, seg[i]] = 1
                    tot[i] = 1
            for k in range(E):
                e = orders[r, k]
                a = e1[e]
                while parent[a] != a:
                    parent[a] = parent[parent[a]]
                    a = parent[a]
                b = e2[e]
                while parent[b] != b:
                    parent[b] = parent[parent[b]]
                    b = parent[b]
                if a == b:
                    continue
                common = np.int64(0)
                for l in range(1, L + 1):
                    common += cnt[a, l] * cnt[b, l]
                if pos:
                    w[r, e] = common
                else:
                    w[r, e] = tot[a] * tot[b] - common
                for l in range(1, L + 1):
                    cnt[b, l] += cnt[a, l]
                tot[b] += tot[a]
                parent[a] = b
        return w

    _malis_runs = _malis_runs_nb
except Exception:  # pragma: no cover
    _malis_runs = None


def malis_weights_full(pred, target):
    """Bit-exact clone of the reference's _malis_weights_full."""
    pred = np.ascontiguousarray(np.asarray(pred, dtype=np.float32))
    target = np.ascontiguousarray(np.asarray(target, dtype=np.float32))
    B, C, H, W = pred.shape
    nR, nC = H // WIN, W // WIN
    R = B * nR * nC

    p = pred[:, 0].reshape(B, nR, WIN, nC, WIN).transpose(0, 1, 3, 2, 4)
    t = target[:, 0].reshape(B, nR, WIN, nC, WIN).transpose(0, 1, 3, 2, 4)

    costs_h = (p[..., :, :-1] + p[..., :, 1:]).reshape(B, nR, nC, _HALF)
    costs_v = (p[..., :-1, :] + p[..., 1:, :]).reshape(B, nR, nC, _HALF)
    costs = np.concatenate([costs_h, costs_v], axis=-1)  # (B,nR,nC,E) f32
    gt_h = (t[..., :, :-1] + t[..., :, 1:]).reshape(B, nR, nC, _HALF)
    gt_v = (t[..., :-1, :] + t[..., 1:, :]).reshape(B, nR, nC, _HALF)
    gt = np.concatenate([gt_h, gt_v], axis=-1)

    costs_n = costs.copy()
    costs_p = costs.copy()
    costs_n[gt > 20] = 20
    costs_p[gt < 10] = 0
    gtc = np.minimum(gt, 20)

    # stable descending argsort — identical tie-breaking to the reference
    order_n = np.ascontiguousarray(
        np.argsort(-costs_n, axis=-1, kind="stable").reshape(R, _E)
    )
    order_p = np.ascontiguousarray(
        np.argsort(-costs_p, axis=-1, kind="stable").reshape(R, _E)
    )

    segs = np.empty((B, nR, nC, _N), np.int32)
    bg = t == 0.0
    for b in range(B):
        for r in range(nR):
            for c in range(nC):
                segs[b, r, c] = _label_bg(bg[b, r, c]).ravel()
    segs2 = segs.reshape(R, _N)

    global _malis_runs
    if _malis_runs is not None:
        try:
            wn = _malis_runs(order_n, segs2, E1, E2, 0)
            wp = _malis_runs(order_p, segs2, E1, E2, 1)
        except Exception:
            _malis_runs = None
            wn = _malis_runs_py(order_n, segs2, E1, E2, 0)
            wp = _malis_runs_py(order_p, segs2, E1, E2, 1)
    else:
        wn = _malis_runs_py(order_n, segs2, E1, E2, 0)
        wp = _malis_runs_py(order_p, segs2, E1, E2, 1)

    out = []
    gtc_flat = gtc.reshape(R, _E)
    for w, is_pos in ((wn, False), (wp, True)):
        w64 = w.astype(np.float64)
        s = w64.sum(axis=-1, keepdims=True)
        np.divide(w64, s, out=w64, where=s > 0)
        if is_pos:
            w64[gtc_flat < 20] = 0
        else:
            w64[gtc_flat >= 10] = 0
        wh = w64[:, :_HALF].reshape(R, WIN, WIN - 1)
        wv = w64[:, _HALF:].reshape(R, WIN - 1, WIN)
        nw = np.zeros((R, WIN, WIN), np.float64)
        nw[:, :, :-1] += wh
        nw[:, :, 1:] += wh
        nw[:, :-1, :] += wv
        nw[:, 1:, :] += wv
        img = (
            nw.reshape(B, nR, nC, WIN, WIN)
            .transpose(0, 1, 3, 2, 4)
            .reshape(B, 1, H, W)
            .astype(np.float32)
        )
        out.append(img)
    return out[0], out[1]


# ===========================================================================
# Device side: weighted-loss reduction on 8 NeuronCores.
# ===========================================================================

N_CORES = 8
_P = 128  # SBUF partitions
_TOT = 4 * 1 * 256 * 256  # 262144 pixels
_PER_CORE = _TOT // N_CORES  # 32768
_F = _PER_CORE // _P  # 256 floats per partition per tensor

_NC_CACHE = {}


def _build_nc():
    import concourse.bacc as bacc
    import concourse.tile as tile
    from concourse import mybir

    f32 = mybir.dt.float32
    nc = bacc.Bacc("TRN2", target_bir_lowering=False)
    yp_d = nc.dram_tensor("yp", [_P, _F], f32, kind="ExternalInput")
    wn_d = nc.dram_tensor("wn", [_P, _F], f32, kind="ExternalInput")
    wp_d = nc.dram_tensor("wp", [_P, _F], f32, kind="ExternalInput")
    out_d = nc.dram_tensor("out", [1, 1], f32, kind="ExternalOutput")

    with tile.TileContext(nc) as tc:
        with (
            tc.tile_pool(name="io", bufs=1) as io,
            tc.tile_pool(name="ps", bufs=1, space="PSUM") as ps,
        ):
            # three input loads on three different engine queues (parallel)
            t_yp = io.tile([_P, _F], f32)
            t_wn = io.tile([_P, _F], f32)
            t_wp = io.tile([_P, _F], f32)
            nc.sync.dma_start(out=t_yp[:, :], in_=yp_d[:, :])
            nc.scalar.dma_start(out=t_wn[:, :], in_=wn_d[:, :])
            nc.gpsimd.dma_start(out=t_wp[:, :], in_=wp_d[:, :])

            ones = io.tile([_P, 1], f32)
            nc.gpsimd.memset(ones[:, :], 1.0)

            # DVE chain; yp-only ops first so they overlap the wn/wp loads
            t_sq1 = io.tile([_P, _F], f32)
            nc.vector.tensor_mul(t_sq1[:, :], t_yp[:, :], t_yp[:, :])
            t_b = io.tile([_P, _F], f32)
            nc.vector.tensor_scalar_sub(t_b[:, :], t_yp[:, :], 20.0)
            t_sq2 = io.tile([_P, _F], f32)
            nc.vector.tensor_mul(t_sq2[:, :], t_b[:, :], t_b[:, :])
            t_m1 = io.tile([_P, _F], f32)
            nc.vector.tensor_mul(t_m1[:, :], t_sq1[:, :], t_wn[:, :])
            t_m2 = io.tile([_P, _F], f32)
            nc.vector.tensor_mul(t_m2[:, :], t_sq2[:, :], t_wp[:, :])

            # column sums via PE: psum[1,256] = ones.T @ m1 + ones.T @ m2
            col = ps.tile([1, _F], f32)
            nc.tensor.matmul(col[:, :], ones[:, :], t_m1[:, :], start=True, stop=False)
            nc.tensor.matmul(col[:, :], ones[:, :], t_m2[:, :], start=False, stop=True)

            # final reduce [1,256] -> [1,1] and 4-byte store
            t_out = io.tile([1, 1], f32)
            nc.vector.reduce_sum(
                out=t_out[:, :], in_=col[:, :], axis=mybir.AxisListType.X
            )
            nc.sync.dma_start(out=out_d[:, :], in_=t_out[:, :])
    nc.finalize()
    return nc


import os as _os

# The final walrus-emitted SP epilogue DRAIN waits out the DMA ring, so the
# explicit post-issue semaphore wait on the 4-byte output store is redundant;
# validated correct across repeated 8-core runs. K_DROP_OUT_WAIT=0 restores it.
_DROP_OUT_WAIT = _os.environ.get("K_DROP_OUT_WAIT", "1") == "1"


def _build_nc_raw3(surgery=False):
    """v3: single packed input X=[128,768] (yp | A=wn+wp | B=-40*wp) loaded
    as two partition-halves on the two HWDGE queues (sync+scalar),
    Horner form loss_e=(A*y+B)*y; the constant term 400*wp is corrected on
    host via -10*sum(B). Raw Bacc, manual semaphores."""
    from contextlib import ExitStack

    import concourse.bacc as bacc
    from concourse import mybir

    f32 = mybir.dt.float32
    nc = bacc.Bacc("TRN2", target_bir_lowering=False)
    x_d = nc.dram_tensor("x", [_P, 3 * _F], f32, kind="ExternalInput")
    out_d = nc.dram_tensor("out", [1, 1], f32, kind="ExternalOutput")

    with ExitStack() as ctx:
        t_x = ctx.enter_context(nc.sbuf_tensor([_P, 3 * _F], f32))
        t_1 = ctx.enter_context(nc.sbuf_tensor([_P, _F], f32))
        t_2 = ctx.enter_context(nc.sbuf_tensor([_P, _F], f32))
        t_3 = ctx.enter_context(nc.sbuf_tensor([_P, _F], f32))
        ones = ctx.enter_context(nc.sbuf_tensor([_P, 1], f32))
        p_t = ctx.enter_context(nc.sbuf_tensor([_P, 1], f32))
        t_out = ctx.enter_context(nc.sbuf_tensor([1, 1], f32))
        acc = ctx.enter_context(nc.psum_tensor([1, 1], f32))

        s_x = ctx.enter_context(nc.semaphore("s_x"))
        s_ones = ctx.enter_context(nc.semaphore("s_ones"))
        s_p = ctx.enter_context(nc.semaphore("s_p"))
        s_mm = ctx.enter_context(nc.semaphore("s_mm"))
        s_res = ctx.enter_context(nc.semaphore("s_res"))
        s_out = ctx.enter_context(nc.semaphore("s_out"))
        s_v = ctx.enter_context(nc.semaphore("s_v"))
        block = ctx.enter_context(nc.Block())

        yp = t_x[:, 0:_F]
        A = t_x[:, _F : 2 * _F]
        B = t_x[:, 2 * _F : 3 * _F]
        H = _P // 2

        @block.sync
        def _(sync):
            sync.dma_start(out=t_x[:H, :], in_=x_d[:H, :]).then_inc(s_x, 16)
            sync.wait_ge(s_res, 1)
            sync.dma_start(out=out_d[:, :], in_=t_out[:, :]).then_inc(s_out, 16)
            if not _DROP_OUT_WAIT:
                sync.wait_ge(s_out, 16)

        @block.scalar
        def _(scalar):
            scalar.dma_start(out=t_x[H:, :], in_=x_d[H:, :]).then_inc(s_x, 16)

        @block.gpsimd
        def _(gpsimd):
            gpsimd.memset(ones[:, :], 1.0).then_inc(s_ones, 1)

        @block.vector
        def _(vector):
            n = [0]

            def step(ins):
                n[0] += 1
                ins.then_inc(s_v, 1)

            def fence():
                vector.wait_ge(s_v, n[0])

            vector.wait_ge(s_x, 32)
            step(nc.vector.tensor_mul(t_1[:, :], A, yp))
            fence()
            step(nc.vector.tensor_add(t_2[:, :], t_1[:, :], B))
            fence()
            step(nc.vector.tensor_mul(t_3[:, :], t_2[:, :], yp))
            fence()
            nc.vector.reduce_sum(
                out=p_t[:, :], in_=t_3[:, :], axis=mybir.AxisListType.X
            ).then_inc(s_p, 1)
            vector.wait_ge(s_mm, 1)
            nc.vector.tensor_copy(t_out[:, :], acc[:, :]).then_inc(s_res, 1)

        @block.tensor
        def _(tensor):
            tensor.wait_ge(s_p, 1)
            tensor.wait_ge(s_ones, 1)
            nc.tensor.matmul(
                acc[:, :], p_t[:, :], ones[:, :], start=True, stop=True
            ).then_inc(s_mm, 1)

    if surgery:
        _strip_barriers(nc)
    nc.finalize()
    return nc


def _strip_barriers(nc):
    """Remove the main-block entry barrier round, the unused const-AP
    memsets, and the Block-end barrier round. Only touches the framework's
    prologue/epilogue blocks; cross-engine deps in the engine blocks are
    fully covered by explicit semaphores."""
    from concourse import mybir

    for bb in nc.main_func.blocks:
        if bb.name != "main" and not bb.name.endswith("_end"):
            continue
        keep = []
        for ins in bb.instructions:
            if isinstance(ins, (mybir.InstDrain, mybir.InstEventSemaphore)):
                continue
            if bb.name == "main" and isinstance(ins, mybir.InstMemset):
                outs = getattr(ins, "outs", [])
                names = str(outs)
                if "const-" in names:
                    continue
            keep.append(ins)
        bb.instructions[:] = keep


def _build_nc_raw():
    """Raw Bacc kernel with manual semaphores — skips the Tile framework's
    entry/exit all-engine barriers, ordering modes, and extra prologue."""
    from contextlib import ExitStack

    import concourse.bacc as bacc
    from concourse import mybir

    f32 = mybir.dt.float32
    nc = bacc.Bacc("TRN2", target_bir_lowering=False)
    yp_d = nc.dram_tensor("yp", [_P, _F], f32, kind="ExternalInput")
    wn_d = nc.dram_tensor("wn", [_P, _F], f32, kind="ExternalInput")
    wp_d = nc.dram_tensor("wp", [_P, _F], f32, kind="ExternalInput")
    out_d = nc.dram_tensor("out", [1, 1], f32, kind="ExternalOutput")

    with ExitStack() as ctx:
        t_yp = ctx.enter_context(nc.sbuf_tensor([_P, _F], f32))
        t_wn = ctx.enter_context(nc.sbuf_tensor([_P, _F], f32))
        t_wp = ctx.enter_context(nc.sbuf_tensor([_P, _F], f32))
        t_sq1 = ctx.enter_context(nc.sbuf_tensor([_P, _F], f32))
        t_b = ctx.enter_context(nc.sbuf_tensor([_P, _F], f32))
        t_sq2 = ctx.enter_context(nc.sbuf_tensor([_P, _F], f32))
        t_m1 = ctx.enter_context(nc.sbuf_tensor([_P, _F], f32))
        t_m2 = ctx.enter_context(nc.sbuf_tensor([_P, _F], f32))
        ones = ctx.enter_context(nc.sbuf_tensor([_P, 1], f32))
        p_a = ctx.enter_context(nc.sbuf_tensor([_P, 1], f32))
        p_b = ctx.enter_context(nc.sbuf_tensor([_P, 1], f32))
        p_t = ctx.enter_context(nc.sbuf_tensor([_P, 1], f32))
        t_out = ctx.enter_context(nc.sbuf_tensor([1, 1], f32))
        acc = ctx.enter_context(nc.psum_tensor([1, 1], f32))

        s_yp = ctx.enter_context(nc.semaphore("s_yp"))
        s_wn = ctx.enter_context(nc.semaphore("s_wn"))
        s_wp = ctx.enter_context(nc.semaphore("s_wp"))
        s_ones = ctx.enter_context(nc.semaphore("s_ones"))
        s_p = ctx.enter_context(nc.semaphore("s_p"))
        s_mm = ctx.enter_context(nc.semaphore("s_mm"))
        s_res = ctx.enter_context(nc.semaphore("s_res"))
        s_out = ctx.enter_context(nc.semaphore("s_out"))
        s_v = ctx.enter_context(nc.semaphore("s_v"))
        block = ctx.enter_context(nc.Block())

        @block.sync
        def _(sync):
            sync.dma_start(out=t_yp[:, :], in_=yp_d[:, :]).then_inc(s_yp, 16)
            sync.wait_ge(s_res, 1)
            sync.dma_start(out=out_d[:, :], in_=t_out[:, :]).then_inc(s_out, 16)
            sync.wait_ge(s_out, 16)

        @block.scalar
        def _(scalar):
            scalar.dma_start(out=t_wn[:, :], in_=wn_d[:, :]).then_inc(s_wn, 16)

        @block.gpsimd
        def _(gpsimd):
            gpsimd.memset(ones[:, :], 1.0).then_inc(s_ones, 1)
            gpsimd.dma_start(out=t_wp[:, :], in_=wp_d[:, :]).then_inc(s_wp, 16)

        @block.vector
        def _(vector):
            # s_v serializes the DVE RAW chains (the engine pipeline does
            # not interlock same-engine SBUF read-after-write).
            n = [0]

            def step(ins):
                n[0] += 1
                ins.then_inc(s_v, 1)

            def fence():
                vector.wait_ge(s_v, n[0])

            vector.wait_ge(s_yp, 16)
            step(nc.vector.tensor_mul(t_sq1[:, :], t_yp[:, :], t_yp[:, :]))
            step(nc.vector.tensor_scalar_sub(t_b[:, :], t_yp[:, :], 20.0))
            fence()
            step(nc.vector.tensor_mul(t_sq2[:, :], t_b[:, :], t_b[:, :]))
            vector.wait_ge(s_wn, 16)
            fence()
            step(nc.vector.tensor_mul(t_m1[:, :], t_sq1[:, :], t_wn[:, :]))
            fence()
            step(
                nc.vector.reduce_sum(
                    out=p_a[:, :], in_=t_m1[:, :], axis=mybir.AxisListType.X
                )
            )
            vector.wait_ge(s_wp, 16)
            step(nc.vector.tensor_mul(t_m2[:, :], t_sq2[:, :], t_wp[:, :]))
            fence()
            step(
                nc.vector.reduce_sum(
                    out=p_b[:, :], in_=t_m2[:, :], axis=mybir.AxisListType.X
                )
            )
            fence()
            nc.vector.tensor_add(p_t[:, :], p_a[:, :], p_b[:, :]).then_inc(s_p, 1)
            vector.wait_ge(s_mm, 1)
            nc.vector.tensor_copy(t_out[:, :], acc[:, :]).then_inc(s_res, 1)

        @block.tensor
        def _(tensor):
            tensor.wait_ge(s_p, 1)
            tensor.wait_ge(s_ones, 1)
            nc.tensor.matmul(
                acc[:, :], p_t[:, :], ones[:, :], start=True, stop=True
            ).then_inc(s_mm, 1)

    nc.finalize()
    return nc


def _impl():
    import os

    return os.environ.get("K_IMPL", "raw3s")


def _get_nc():
    if "nc" not in _NC_CACHE:
        impl = _impl()
        if impl == "tile":
            _NC_CACHE["nc"] = _build_nc()
        elif impl == "raw":
            _NC_CACHE["nc"] = _build_nc_raw()
        elif impl == "raw3":
            _NC_CACHE["nc"] = _build_nc_raw3(surgery=False)
        else:
            _NC_CACHE["nc"] = _build_nc_raw3(surgery=True)
    return _NC_CACHE["nc"]


def _shard(arr):
    """(4,1,256,256) f32 -> list of 8 [128, 256] per-core chunks."""
    flat = np.ascontiguousarray(arr, dtype=np.float32).reshape(N_CORES, _P, _F)
    return [flat[c] for c in range(N_CORES)]


def run_device(y_pred, wn_img, wp_img, trace=False, **kw):
    from concourse.bass_utils import run_bass_kernel_spmd

    nc = _get_nc()
    impl = _impl()
    if impl in ("tile", "raw"):
        yps = _shard(y_pred)
        wns = _shard(wn_img)
        wps = _shard(wp_img)
        in_maps = [
            {"yp": yps[c], "wn": wns[c], "wp": wps[c]} for c in range(N_CORES)
        ]
        correction = 0.0
    else:
        A = wn_img + wp_img
        B = wp_img * np.float32(-40.0)
        xs = np.concatenate(
            [
                np.ascontiguousarray(y_pred, dtype=np.float32).reshape(
                    N_CORES, _P, _F
                ),
                A.reshape(N_CORES, _P, _F),
                B.reshape(N_CORES, _P, _F),
            ],
            axis=2,
        )
        in_maps = [{"x": np.ascontiguousarray(xs[c])} for c in range(N_CORES)]
        # device returns sum((A*y+B)*y); the constant term 400*wp == -10*B
        correction = -10.0 * B.astype(np.float64).sum()
    res = run_bass_kernel_spmd(
        nc, in_maps, core_ids=list(range(N_CORES)), trace=trace, **kw
    )
    partials = np.array(
        [float(res.results[c]["out"][0, 0]) for c in range(N_CORES)], dtype=np.float64
    )
    total = np.float32(partials.sum() + correction)
    return total, res


def kernel(y_true, y_pred):
    y_true = np.asarray(y_true, dtype=np.float32)
    y_pred = np.asarray(y_pred, dtype=np.float32)
    wn_img, wp_img = malis_weights_full(y_pred, y_true)
    total, _ = run_device(y_pred, wn_img, wp_img, trace=False)
    return np.array(total, dtype=np.float32)


# revision 25
# speedup vs baseline: 1.7513x; 1.0146x over previous
"""Trainium2 kernel for nn_ConnectivityLoss (MALIS connectivity loss).

Contract: kernel(y_true, y_pred) -> scalar float32 loss, matching

    loss = sum(y_pred**2 * weights_n + (20 - y_pred)**2 * weights_p)

where weights_n / weights_p are the MALIS maximin edge weights computed per
32x32 window.  The reference itself computes the weights with a host
``jax.pure_callback`` (sequential Kruskal + union-find is not device work);
this kernel mirrors that split: a fast bit-exact host implementation of the
weights, and the memory-bound weighted reduction on 8 NeuronCores
(data-parallel over the flattened pixels).
"""

import numpy as np

# ===========================================================================
# Host side: bit-exact fast reimplementation of the reference MALIS weights.
# ===========================================================================

WIN = 32
_N = WIN * WIN
_idx = np.arange(_N).reshape(WIN, WIN)
E1 = np.concatenate([_idx[:, :-1].ravel(), _idx[:-1, :].ravel()]).astype(np.int64)
E2 = np.concatenate([_idx[:, 1:].ravel(), _idx[1:, :].ravel()]).astype(np.int64)
_E = E1.shape[0]  # 1984
_HALF = WIN * (WIN - 1)  # 992

try:
    from scipy import ndimage as _ndi

    def _label_bg(bg):  # bg: bool (WIN, WIN) -> int32 labels, 0 = unlabeled
        lab, _ = _ndi.label(bg)
        return lab.astype(np.int32)
except Exception:  # pragma: no cover

    def _label_bg(bg):
        lab = np.zeros((WIN, WIN), np.int32)
        nxt = 0
        stack = []
        for i in range(WIN):
            for j in range(WIN):
                if bg[i, j] and lab[i, j] == 0:
                    nxt += 1
                    stack.append((i, j))
                    lab[i, j] = nxt
                    while stack:
                        a, b = stack.pop()
                        for u, v in ((a - 1, b), (a + 1, b), (a, b - 1), (a, b + 1)):
                            if 0 <= u < WIN and 0 <= v < WIN and bg[u, v] and lab[u, v] == 0:
                                lab[u, v] = nxt
                                stack.append((u, v))
        return lab


def _malis_runs_py(orders, segs, e1, e2, pos):
    """Pure-python fallback: Kruskal maximin MALIS weighting, R runs."""
    R, E = orders.shape
    N = segs.shape[1]
    w = np.zeros((R, E), np.int64)
    for r in range(R):
        seg = segs[r]
        parent = np.arange(N, dtype=np.int64)
        cnts = [({int(seg[i]): 1} if seg[i] != 0 else {}) for i in range(N)]
        tot = [len(c) for c in cnts]
        wr = w[r]
        order = orders[r]
        for k in range(E):
            e = order[k]
            a = e1[e]
            while parent[a] != a:
                parent[a] = parent[parent[a]]
                a = parent[a]
            b = e2[e]
            while parent[b] != b:
                parent[b] = parent[parent[b]]
                b = parent[b]
            if a == b:
                continue
            ca, cb = cnts[a], cnts[b]
            if len(ca) > len(cb):
                a, b, ca, cb = b, a, cb, ca
            common = 0
            for l, c in ca.items():
                if l in cb:
                    common += c * cb[l]
            if pos:
                wr[e] = common
            else:
                wr[e] = tot[a] * tot[b] - common
            for l, c in ca.items():
                cb[l] = cb.get(l, 0) + c
            tot[b] += tot[a]
            parent[a] = b
            cnts[a] = {}
    return w


_malis_runs = None
try:
    import numba

    @numba.njit(cache=True)
    def _malis_runs_nb(orders, segs, e1, e2, pos):
        R, E = orders.shape
        N = segs.shape[1]
        w = np.zeros((R, E), np.int64)
        for r in range(R):
            seg = segs[r]
            L = 0
            for i in range(N):
                if seg[i] > L:
                    L = seg[i]
            parent = np.arange(N, dtype=np.int64)
            cnt = np.zeros((N, L + 1), np.int64)
            tot = np.zeros(N, np.int64)
            for i in range(N):
                if seg[i] != 0:
                    cnt[i, seg[i]] = 1
                    tot[i] = 1
            for k in range(E):
                e = orders[r, k]
                a = e1[e]
                while parent[a] != a:
                    parent[a] = parent[parent[a]]
                    a = parent[a]
                b = e2[e]
                while parent[b] != b:
                    parent[b] = parent[parent[b]]
                    b = parent[b]
                if a == b:
                    continue
                common = np.int64(0)
                for l in range(1, L + 1):
                    common += cnt[a, l] * cnt[b, l]
                if pos:
                    w[r, e] = common
                else:
                    w[r, e] = tot[a] * tot[b] - common
                for l in range(1, L + 1):
                    cnt[b, l] += cnt[a, l]
                tot[b] += tot[a]
                parent[a] = b
        return w

    _malis_runs = _malis_runs_nb
except Exception:  # pragma: no cover
    _malis_runs = None


def malis_weights_full(pred, target):
    """Bit-exact clone of the reference's _malis_weights_full."""
    pred = np.ascontiguousarray(np.asarray(pred, dtype=np.float32))
    target = np.ascontiguousarray(np.asarray(target, dtype=np.float32))
    B, C, H, W = pred.shape
    nR, nC = H // WIN, W // WIN
    R = B * nR * nC

    p = pred[:, 0].reshape(B, nR, WIN, nC, WIN).transpose(0, 1, 3, 2, 4)
    t = target[:, 0].reshape(B, nR, WIN, nC, WIN).transpose(0, 1, 3, 2, 4)

    costs_h = (p[..., :, :-1] + p[..., :, 1:]).reshape(B, nR, nC, _HALF)
    costs_v = (p[..., :-1, :] + p[..., 1:, :]).reshape(B, nR, nC, _HALF)
    costs = np.concatenate([costs_h, costs_v], axis=-1)  # (B,nR,nC,E) f32
    gt_h = (t[..., :, :-1] + t[..., :, 1:]).reshape(B, nR, nC, _HALF)
    gt_v = (t[..., :-1, :] + t[..., 1:, :]).reshape(B, nR, nC, _HALF)
    gt = np.concatenate([gt_h, gt_v], axis=-1)

    costs_n = costs.copy()
    costs_p = costs.copy()
    costs_n[gt > 20] = 20
    costs_p[gt < 10] = 0
    gtc = np.minimum(gt, 20)

    # stable descending argsort — identical tie-breaking to the reference
    order_n = np.ascontiguousarray(
        np.argsort(-costs_n, axis=-1, kind="stable").reshape(R, _E)
    )
    order_p = np.ascontiguousarray(
        np.argsort(-costs_p, axis=-1, kind="stable").reshape(R, _E)
    )

    segs = np.empty((B, nR, nC, _N), np.int32)
    bg = t == 0.0
    for b in range(B):
        for r in range(nR):
            for c in range(nC):
                segs[b, r, c] = _label_bg(bg[b, r, c]).ravel()
    segs2 = segs.reshape(R, _N)

    global _malis_runs
    if _malis_runs is not None:
        try:
            wn = _malis_runs(order_n, segs2, E1, E2, 0)
            wp = _malis_runs(order_p, segs2, E1, E2, 1)
        except Exception:
            _malis_runs = None
            wn = _malis_runs_py(order_n, segs2, E1, E2, 0)
            wp = _malis_runs_py(order_p, segs2, E1, E2, 1)
    else:
        wn = _malis_runs_py(order_n, segs2, E1, E2, 0)
        wp = _malis_runs_py(order_p, segs2, E1, E2, 1)

    out = []
    gtc_flat = gtc.reshape(R, _E)
    for w, is_pos in ((wn, False), (wp, True)):
        w64 = w.astype(np.float64)
        s = w64.sum(axis=-1, keepdims=True)
        np.divide(w64, s, out=w64, where=s > 0)
        if is_pos:
            w64[gtc_flat < 20] = 0
        else:
            w64[gtc_flat >= 10] = 0
        wh = w64[:, :_HALF].reshape(R, WIN, WIN - 1)
        wv = w64[:, _HALF:].reshape(R, WIN - 1, WIN)
        nw = np.zeros((R, WIN, WIN), np.float64)
        nw[:, :, :-1] += wh
        nw[:, :, 1:] += wh
        nw[:, :-1, :] += wv
        nw[:, 1:, :] += wv
        img = (
            nw.reshape(B, nR, nC, WIN, WIN)
            .transpose(0, 1, 3, 2, 4)
            .reshape(B, 1, H, W)
            .astype(np.float32)
        )
        out.append(img)
    return out[0], out[1]


# ===========================================================================
# Device side: weighted-loss reduction on 8 NeuronCores.
# ===========================================================================

N_CORES = 8
_P = 128  # SBUF partitions
_TOT = 4 * 1 * 256 * 256  # 262144 pixels
_PER_CORE = _TOT // N_CORES  # 32768
_F = _PER_CORE // _P  # 256 floats per partition per tensor

_NC_CACHE = {}


def _build_nc():
    import concourse.bacc as bacc
    import concourse.tile as tile
    from concourse import mybir

    f32 = mybir.dt.float32
    nc = bacc.Bacc("TRN2", target_bir_lowering=False)
    yp_d = nc.dram_tensor("yp", [_P, _F], f32, kind="ExternalInput")
    wn_d = nc.dram_tensor("wn", [_P, _F], f32, kind="ExternalInput")
    wp_d = nc.dram_tensor("wp", [_P, _F], f32, kind="ExternalInput")
    out_d = nc.dram_tensor("out", [1, 1], f32, kind="ExternalOutput")

    with tile.TileContext(nc) as tc:
        with (
            tc.tile_pool(name="io", bufs=1) as io,
            tc.tile_pool(name="ps", bufs=1, space="PSUM") as ps,
        ):
            # three input loads on three different engine queues (parallel)
            t_yp = io.tile([_P, _F], f32)
            t_wn = io.tile([_P, _F], f32)
            t_wp = io.tile([_P, _F], f32)
            nc.sync.dma_start(out=t_yp[:, :], in_=yp_d[:, :])
            nc.scalar.dma_start(out=t_wn[:, :], in_=wn_d[:, :])
            nc.gpsimd.dma_start(out=t_wp[:, :], in_=wp_d[:, :])

            ones = io.tile([_P, 1], f32)
            nc.gpsimd.memset(ones[:, :], 1.0)

            # DVE chain; yp-only ops first so they overlap the wn/wp loads
            t_sq1 = io.tile([_P, _F], f32)
            nc.vector.tensor_mul(t_sq1[:, :], t_yp[:, :], t_yp[:, :])
            t_b = io.tile([_P, _F], f32)
            nc.vector.tensor_scalar_sub(t_b[:, :], t_yp[:, :], 20.0)
            t_sq2 = io.tile([_P, _F], f32)
            nc.vector.tensor_mul(t_sq2[:, :], t_b[:, :], t_b[:, :])
            t_m1 = io.tile([_P, _F], f32)
            nc.vector.tensor_mul(t_m1[:, :], t_sq1[:, :], t_wn[:, :])
            t_m2 = io.tile([_P, _F], f32)
            nc.vector.tensor_mul(t_m2[:, :], t_sq2[:, :], t_wp[:, :])

            # column sums via PE: psum[1,256] = ones.T @ m1 + ones.T @ m2
            col = ps.tile([1, _F], f32)
            nc.tensor.matmul(col[:, :], ones[:, :], t_m1[:, :], start=True, stop=False)
            nc.tensor.matmul(col[:, :], ones[:, :], t_m2[:, :], start=False, stop=True)

            # final reduce [1,256] -> [1,1] and 4-byte store
            t_out = io.tile([1, 1], f32)
            nc.vector.reduce_sum(
                out=t_out[:, :], in_=col[:, :], axis=mybir.AxisListType.X
            )
            nc.sync.dma_start(out=out_d[:, :], in_=t_out[:, :])
    nc.finalize()
    return nc


import os as _os

# The final walrus-emitted SP epilogue DRAIN waits out the DMA ring, so the
# explicit post-issue semaphore wait on the 4-byte output store is redundant;
# validated correct across repeated 8-core runs. K_DROP_OUT_WAIT=0 restores it.
_DROP_OUT_WAIT = _os.environ.get("K_DROP_OUT_WAIT", "1") == "1"


def _build_nc_raw3(surgery=False):
    """v3: single packed input X=[128,768] (yp | A=wn+wp | B=-40*wp) loaded
    as two partition-halves on the two HWDGE queues (sync+scalar),
    Horner form loss_e=(A*y+B)*y; the constant term 400*wp is corrected on
    host via -10*sum(B). Raw Bacc, manual semaphores."""
    from contextlib import ExitStack

    import concourse.bacc as bacc
    from concourse import mybir

    f32 = mybir.dt.float32
    nc = bacc.Bacc("TRN2", target_bir_lowering=False)
    x_d = nc.dram_tensor("x", [_P, 3 * _F], f32, kind="ExternalInput")
    out_d = nc.dram_tensor("out", [1, 1], f32, kind="ExternalOutput")

    with ExitStack() as ctx:
        t_x = ctx.enter_context(nc.sbuf_tensor([_P, 3 * _F], f32))
        t_1 = ctx.enter_context(nc.sbuf_tensor([_P, _F], f32))
        t_2 = ctx.enter_context(nc.sbuf_tensor([_P, _F], f32))
        t_3 = ctx.enter_context(nc.sbuf_tensor([_P, _F], f32))
        ones = ctx.enter_context(nc.sbuf_tensor([_P, 1], f32))
        p_t = ctx.enter_context(nc.sbuf_tensor([_P, 1], f32))
        t_out = ctx.enter_context(nc.sbuf_tensor([1, 1], f32))
        acc = ctx.enter_context(nc.psum_tensor([1, 1], f32))

        s_x = ctx.enter_context(nc.semaphore("s_x"))
        s_ones = ctx.enter_context(nc.semaphore("s_ones"))
        s_p = ctx.enter_context(nc.semaphore("s_p"))
        s_mm = ctx.enter_context(nc.semaphore("s_mm"))
        s_res = ctx.enter_context(nc.semaphore("s_res"))
        s_out = ctx.enter_context(nc.semaphore("s_out"))
        s_v = ctx.enter_context(nc.semaphore("s_v"))
        block = ctx.enter_context(nc.Block())

        yp = t_x[:, 0:_F]
        A = t_x[:, _F : 2 * _F]
        B = t_x[:, 2 * _F : 3 * _F]
        H = _P // 2

        @block.sync
        def _(sync):
            sync.dma_start(out=t_x[:H, :], in_=x_d[:H, :]).then_inc(s_x, 16)
            sync.wait_ge(s_res, 1)
            sync.dma_start(out=out_d[:, :], in_=t_out[:, :]).then_inc(s_out, 16)
            if not _DROP_OUT_WAIT:
                sync.wait_ge(s_out, 16)

        @block.scalar
        def _(scalar):
            scalar.dma_start(out=t_x[H:, :], in_=x_d[H:, :]).then_inc(s_x, 16)

        @block.gpsimd
        def _(gpsimd):
            gpsimd.memset(ones[:, :], 1.0).then_inc(s_ones, 1)

        @block.vector
        def _(vector):
            n = [0]

            def step(ins):
                n[0] += 1
                ins.then_inc(s_v, 1)

            def fence():
                vector.wait_ge(s_v, n[0])

            vector.wait_ge(s_x, 32)
            step(nc.vector.tensor_mul(t_1[:, :], A, yp))
            fence()
            step(nc.vector.tensor_add(t_2[:, :], t_1[:, :], B))
            fence()
            step(nc.vector.tensor_mul(t_3[:, :], t_2[:, :], yp))
            fence()
            nc.vector.reduce_sum(
                out=p_t[:, :], in_=t_3[:, :], axis=mybir.AxisListType.X
            ).then_inc(s_p, 1)
            vector.wait_ge(s_mm, 1)
            nc.vector.tensor_copy(t_out[:, :], acc[:, :]).then_inc(s_res, 1)

        @block.tensor
        def _(tensor):
            tensor.wait_ge(s_p, 1)
            tensor.wait_ge(s_ones, 1)
            nc.tensor.matmul(
                acc[:, :], p_t[:, :], ones[:, :], start=True, stop=True
            ).then_inc(s_mm, 1)

    if surgery:
        _strip_barriers(nc)
    nc.finalize()
    return nc


def _build_nc_raw4():
    """v4: like raw3 but with ZERO PE instructions — the cross-partition
    reduce uses a DVE 32x32 block transpose; the output is the 4 transposed
    rows ([4,32] via a partition-strided DMA), summed on host. Removing the
    PE program lets the NEFF entry barrier skip the slow PE init."""
    from contextlib import ExitStack

    import concourse.bass as bass
    import concourse.bacc as bacc
    from concourse import mybir

    f32 = mybir.dt.float32
    nc = bacc.Bacc("TRN2", target_bir_lowering=False)
    x_d = nc.dram_tensor("x", [_P, 3 * _F], f32, kind="ExternalInput")
    out_d = nc.dram_tensor("out", [4, 32], f32, kind="ExternalOutput")

    with ExitStack() as ctx:
        t_x = ctx.enter_context(nc.sbuf_tensor([_P, 3 * _F], f32))
        t_1 = ctx.enter_context(nc.sbuf_tensor([_P, _F], f32))
        t_2 = ctx.enter_context(nc.sbuf_tensor([_P, _F], f32))
        t_3 = ctx.enter_context(nc.sbuf_tensor([_P, _F], f32))
        buf = ctx.enter_context(nc.sbuf_tensor([_P, 32], f32))
        bufT = ctx.enter_context(nc.sbuf_tensor([_P, 32], f32))

        s_x = ctx.enter_context(nc.semaphore("s_x"))
        s_res = ctx.enter_context(nc.semaphore("s_res"))
        s_out = ctx.enter_context(nc.semaphore("s_out"))
        s_v = ctx.enter_context(nc.semaphore("s_v"))
        block = ctx.enter_context(nc.Block())

        yp = t_x[:, 0:_F]
        A = t_x[:, _F : 2 * _F]
        B = t_x[:, 2 * _F : 3 * _F]
        H = _P // 2

        # rows 0/32/64/96 of bufT hold the 128 per-partition sums after the
        # block transpose; gather them with a partition-strided DMA
        gather = bufT[0:128:32, :]

        @block.sync
        def _(sync):
            sync.dma_start(out=t_x[:H, :], in_=x_d[:H, :]).then_inc(s_x, 16)
            sync.wait_ge(s_res, 1)
            sync.dma_start(out=out_d[:, :], in_=gather).then_inc(s_out, 16)
            if not _DROP_OUT_WAIT:
                sync.wait_ge(s_out, 16)

        @block.scalar
        def _(scalar):
            scalar.dma_start(out=t_x[H:, :], in_=x_d[H:, :]).then_inc(s_x, 16)

        @block.vector
        def _(vector):
            n = [0]

            def step(ins):
                n[0] += 1
                ins.then_inc(s_v, 1)

            def fence():
                vector.wait_ge(s_v, n[0])

            step(nc.vector.memset(buf[:, :], 0.0))
            vector.wait_ge(s_x, 32)
            step(nc.vector.tensor_mul(t_1[:, :], A, yp))
            fence()
            step(nc.vector.tensor_add(t_2[:, :], t_1[:, :], B))
            fence()
            step(nc.vector.tensor_mul(t_3[:, :], t_2[:, :], yp))
            fence()
            step(
                nc.vector.reduce_sum(
                    out=buf[:, 0:1], in_=t_3[:, :], axis=mybir.AxisListType.X
                )
            )
            fence()
            nc.vector.transpose(bufT[:, :], buf[:, :]).then_inc(s_res, 1)

    _strip_barriers(nc)
    nc.finalize()
    return nc


def _strip_barriers(nc):
    """Remove the main-block entry barrier round, the unused const-AP
    memsets, and the Block-end barrier round. Only touches the framework's
    prologue/epilogue blocks; cross-engine deps in the engine blocks are
    fully covered by explicit semaphores."""
    from concourse import mybir

    for bb in nc.main_func.blocks:
        if bb.name != "main" and not bb.name.endswith("_end"):
            continue
        keep = []
        for ins in bb.instructions:
            if isinstance(ins, (mybir.InstDrain, mybir.InstEventSemaphore)):
                continue
            if bb.name == "main" and isinstance(ins, mybir.InstMemset):
                outs = getattr(ins, "outs", [])
                names = str(outs)
                if "const-" in names:
                    continue
            keep.append(ins)
        bb.instructions[:] = keep


def _build_nc_raw():
    """Raw Bacc kernel with manual semaphores — skips the Tile framework's
    entry/exit all-engine barriers, ordering modes, and extra prologue."""
    from contextlib import ExitStack

    import concourse.bacc as bacc
    from concourse import mybir

    f32 = mybir.dt.float32
    nc = bacc.Bacc("TRN2", target_bir_lowering=False)
    yp_d = nc.dram_tensor("yp", [_P, _F], f32, kind="ExternalInput")
    wn_d = nc.dram_tensor("wn", [_P, _F], f32, kind="ExternalInput")
    wp_d = nc.dram_tensor("wp", [_P, _F], f32, kind="ExternalInput")
    out_d = nc.dram_tensor("out", [1, 1], f32, kind="ExternalOutput")

    with ExitStack() as ctx:
        t_yp = ctx.enter_context(nc.sbuf_tensor([_P, _F], f32))
        t_wn = ctx.enter_context(nc.sbuf_tensor([_P, _F], f32))
        t_wp = ctx.enter_context(nc.sbuf_tensor([_P, _F], f32))
        t_sq1 = ctx.enter_context(nc.sbuf_tensor([_P, _F], f32))
        t_b = ctx.enter_context(nc.sbuf_tensor([_P, _F], f32))
        t_sq2 = ctx.enter_context(nc.sbuf_tensor([_P, _F], f32))
        t_m1 = ctx.enter_context(nc.sbuf_tensor([_P, _F], f32))
        t_m2 = ctx.enter_context(nc.sbuf_tensor([_P, _F], f32))
        ones = ctx.enter_context(nc.sbuf_tensor([_P, 1], f32))
        p_a = ctx.enter_context(nc.sbuf_tensor([_P, 1], f32))
        p_b = ctx.enter_context(nc.sbuf_tensor([_P, 1], f32))
        p_t = ctx.enter_context(nc.sbuf_tensor([_P, 1], f32))
        t_out = ctx.enter_context(nc.sbuf_tensor([1, 1], f32))
        acc = ctx.enter_context(nc.psum_tensor([1, 1], f32))

        s_yp = ctx.enter_context(nc.semaphore("s_yp"))
        s_wn = ctx.enter_context(nc.semaphore("s_wn"))
        s_wp = ctx.enter_context(nc.semaphore("s_wp"))
        s_ones = ctx.enter_context(nc.semaphore("s_ones"))
        s_p = ctx.enter_context(nc.semaphore("s_p"))
        s_mm = ctx.enter_context(nc.semaphore("s_mm"))
        s_res = ctx.enter_context(nc.semaphore("s_res"))
        s_out = ctx.enter_context(nc.semaphore("s_out"))
        s_v = ctx.enter_context(nc.semaphore("s_v"))
        block = ctx.enter_context(nc.Block())

        @block.sync
        def _(sync):
            sync.dma_start(out=t_yp[:, :], in_=yp_d[:, :]).then_inc(s_yp, 16)
            sync.wait_ge(s_res, 1)
            sync.dma_start(out=out_d[:, :], in_=t_out[:, :]).then_inc(s_out, 16)
            sync.wait_ge(s_out, 16)

        @block.scalar
        def _(scalar):
            scalar.dma_start(out=t_wn[:, :], in_=wn_d[:, :]).then_inc(s_wn, 16)

        @block.gpsimd
        def _(gpsimd):
            gpsimd.memset(ones[:, :], 1.0).then_inc(s_ones, 1)
            gpsimd.dma_start(out=t_wp[:, :], in_=wp_d[:, :]).then_inc(s_wp, 16)

        @block.vector
        def _(vector):
            # s_v serializes the DVE RAW chains (the engine pipeline does
            # not interlock same-engine SBUF read-after-write).
            n = [0]

            def step(ins):
                n[0] += 1
                ins.then_inc(s_v, 1)

            def fence():
                vector.wait_ge(s_v, n[0])

            vector.wait_ge(s_yp, 16)
            step(nc.vector.tensor_mul(t_sq1[:, :], t_yp[:, :], t_yp[:, :]))
            step(nc.vector.tensor_scalar_sub(t_b[:, :], t_yp[:, :], 20.0))
            fence()
            step(nc.vector.tensor_mul(t_sq2[:, :], t_b[:, :], t_b[:, :]))
            vector.wait_ge(s_wn, 16)
            fence()
            step(nc.vector.tensor_mul(t_m1[:, :], t_sq1[:, :], t_wn[:, :]))
            fence()
            step(
                nc.vector.reduce_sum(
                    out=p_a[:, :], in_=t_m1[:, :], axis=mybir.AxisListType.X
                )
            )
            vector.wait_ge(s_wp, 16)
            step(nc.vector.tensor_mul(t_m2[:, :], t_sq2[:, :], t_wp[:, :]))
            fence()
            step(
                nc.vector.reduce_sum(
                    out=p_b[:, :], in_=t_m2[:, :], axis=mybir.AxisListType.X
                )
            )
            fence()
            nc.vector.tensor_add(p_t[:, :], p_a[:, :], p_b[:, :]).then_inc(s_p, 1)
            vector.wait_ge(s_mm, 1)
            nc.vector.tensor_copy(t_out[:, :], acc[:, :]).then_inc(s_res, 1)

        @block.tensor
        def _(tensor):
            tensor.wait_ge(s_p, 1)
            tensor.wait_ge(s_ones, 1)
            nc.tensor.matmul(
                acc[:, :], p_t[:, :], ones[:, :], start=True, stop=True
            ).then_inc(s_mm, 1)

    nc.finalize()
    return nc


def _impl():
    import os

    return os.environ.get("K_IMPL", "raw3s")


def _get_nc():
    if "nc" not in _NC_CACHE:
        impl = _impl()
        if impl == "tile":
            _NC_CACHE["nc"] = _build_nc()
        elif impl == "raw":
            _NC_CACHE["nc"] = _build_nc_raw()
        elif impl == "raw3":
            _NC_CACHE["nc"] = _build_nc_raw3(surgery=False)
        elif impl == "raw4":
            _NC_CACHE["nc"] = _build_nc_raw4()
        else:
            _NC_CACHE["nc"] = _build_nc_raw3(surgery=True)
    return _NC_CACHE["nc"]


def _shard(arr):
    """(4,1,256,256) f32 -> list of 8 [128, 256] per-core chunks."""
    flat = np.ascontiguousarray(arr, dtype=np.float32).reshape(N_CORES, _P, _F)
    return [flat[c] for c in range(N_CORES)]


def run_device(y_pred, wn_img, wp_img, trace=False, **kw):
    from concourse.bass_utils import run_bass_kernel_spmd

    nc = _get_nc()
    impl = _impl()
    if impl in ("tile", "raw"):
        yps = _shard(y_pred)
        wns = _shard(wn_img)
        wps = _shard(wp_img)
        in_maps = [
            {"yp": yps[c], "wn": wns[c], "wp": wps[c]} for c in range(N_CORES)
        ]
        correction = 0.0
    else:
        A = wn_img + wp_img
        B = wp_img * np.float32(-40.0)
        xs = np.concatenate(
            [
                np.ascontiguousarray(y_pred, dtype=np.float32).reshape(
                    N_CORES, _P, _F
                ),
                A.reshape(N_CORES, _P, _F),
                B.reshape(N_CORES, _P, _F),
            ],
            axis=2,
        )
        in_maps = [{"x": np.ascontiguousarray(xs[c])} for c in range(N_CORES)]
        # device returns sum((A*y+B)*y); the constant term 400*wp == -10*B
        correction = -10.0 * B.astype(np.float64).sum()
    res = run_bass_kernel_spmd(
        nc, in_maps, core_ids=list(range(N_CORES)), trace=trace, **kw
    )
    partials = np.array(
        [res.results[c]["out"].astype(np.float64).sum() for c in range(N_CORES)],
        dtype=np.float64,
    )
    total = np.float32(partials.sum() + correction)
    return total, res


def kernel(y_true, y_pred):
    y_true = np.asarray(y_true, dtype=np.float32)
    y_pred = np.asarray(y_pred, dtype=np.float32)
    wn_img, wp_img = malis_weights_full(y_pred, y_true)
    total, _ = run_device(y_pred, wn_img, wp_img, trace=False)
    return np.array(total, dtype=np.float32)
